# revision 28
# baseline (speedup 1.0000x reference)
"""AttnBlock v8: fp8 DoubleRow everywhere, restructured head/tail.

Sharding: core = (batch b in {0,1}) x (query slice s in {0..3}, 1024
queries).  Each core redundantly computes full V^T for its batch
(avoids cross-core collectives), attention for its query slice only.
The host rolls x columns per core so the core's query block is always
columns 0:1024 -- identical program, per-core data.

Math: h = GN(x) = A_c * x + B_c per channel (A, B from runtime stats).
  q = (wq*A)@x + (wq@B + bq)     weight columns scaled on device
  k = (wk*A)@x   (k-bias dropped: per-query-constant under softmax)
  v = (wv*A)@x + const; v-bias folded into the projection bias:
      bp_dev = bp + wp@bv + wp@(wv@B).

v8 over v7 (224 -> 158 -> 153 -> this):
  - half-0 v-production is hoisted before the bias folds / q
    projection: it only depends on wv8 + x8, so the PE warms up (HAM)
    and works while the bias chain resolves on DVE.
  - one PSUM layout for the whole post-stats region (mm 3 + att 3 +
    small 2 = 8 banks); the denominator and the final projection share
    the att pool, so no pool-boundary serialization.
  - v8 psum->fp8 copies split ACT/DVE again (an all-DVE drain at
    ~810ns/tile throttled the v-production below PE pace).
  - acc tiles bf16 (cheaper DVE accumulate + normalize reads).
"""

import os
import sys

import numpy as np

for _p in ("/opt/trn_rl_repo", "/root/.axon_site/_ro/trn_rl_repo"):
    if os.path.isdir(_p) and _p not in sys.path:
        sys.path.insert(0, _p)

B, C, H, W = 2, 512, 64, 64
N = H * W
G = 32
GS = C // G
EPS = 1e-6
NCORES = 8
QS = N // 4               # 1024 queries per core
NHALF = 2                 # key halves
JQ = N // NHALF           # 2048 keys per half
JT = JQ // 128            # 16 key tiles per half
KT2 = JT // 2             # 8 DoubleRow key groups per half
ICH = 512                 # query chunk
NCH = QS // ICH           # 2 chunks
CT = C // 128             # 4 channel tiles
NG = 2                    # DoubleRow channel groups (256 ch each)
SCALE = float(C) ** -0.5
WVS = 16.0                # wv / m0 / wp prescale into fp8
ESHIFT = -3.0             # exp(s + ESHIFT): keep e in fp8e4m3 range
                          # (max scaled score ~7.2; e4m3 max normal 240);
                          # a constant shift cancels in the softmax ratio
NDVE = 6                  # of 8 per-tile stat groups on DVE (rest ACT)

_CACHE = {}


def _build():
    import contextlib

    import concourse.mybir as mybir
    import concourse.tile as tile
    from concourse import bacc
    from concourse.alu_op_type import AluOpType as alu

    f32 = mybir.dt.float32
    bf16 = mybir.dt.bfloat16
    f8 = mybir.dt.float8e4
    AF = mybir.ActivationFunctionType
    PM = mybir.MatmulPerfMode

    nc = bacc.Bacc("TRN2", target_bir_lowering=False, debug=False,
                   num_devices=NCORES)

    xbf = nc.dram_tensor("xbf", [C, N], bf16, kind="ExternalInput").ap()
    x8d = [nc.dram_tensor(f"x8_{g}", [128, 2, N], f8,
                          kind="ExternalInput").ap() for g in range(NG)]
    # wts2 = [m0T | wvT]; wpd = wp.T/16; consts4 = [gamma | beta |
    # bp_eff | qkbc] as columns
    wts2 = nc.dram_tensor("wts2", [C, 2 * C], bf16, kind="ExternalInput").ap()
    wpd = nc.dram_tensor("wpd", [C, C], bf16, kind="ExternalInput").ap()
    consts4 = nc.dram_tensor("consts4", [C, 4], f32,
                             kind="ExternalInput").ap()
    sel = nc.dram_tensor("sel", [128, 8], f32, kind="ExternalInput").ap()
    selT = nc.dram_tensor("selT", [8, 128], f32, kind="ExternalInput").ap()
    ones8 = nc.dram_tensor("ones8", [128, 2, 128], f8,
                           kind="ExternalInput").ap()
    out_d = nc.dram_tensor("out", [C, QS], f32, kind="ExternalOutput").ap()

    def mm(ps, lhsT, rhs, start, stop):
        nc.tensor.matmul(ps, lhsT, rhs, start=start, stop=stop)

    def mm8(ps, lhsT, rhs, start, stop):
        nc.tensor.matmul(ps, lhsT, rhs, start=start, stop=stop,
                         perf_mode=PM.DoubleRow)

    with tile.TileContext(nc) as tc:
        outer = contextlib.ExitStack()
        with outer:
            cpool = outer.enter_context(tc.tile_pool(name="const", bufs=1))
            x_p = outer.enter_context(tc.tile_pool(name="xbf", bufs=1))
            x8_p = outer.enter_context(tc.tile_pool(name="x8", bufs=1))
            acc_p = outer.enter_context(tc.tile_pool(name="acc", bufs=1))
            w_p = outer.enter_context(tc.tile_pool(name="wts", bufs=1))
            q8_p = outer.enter_context(tc.tile_pool(name="q8", bufs=1))
            v8_p = outer.enter_context(tc.tile_pool(name="v8", bufs=2 * KT2))
            e8_p = outer.enter_context(tc.tile_pool(name="e8",
                                                    bufs=2 * KT2 + 2))
            f_p = outer.enter_context(tc.tile_pool(name="fin", bufs=1))

            # ---- Sync ring, in transfer-priority order: x chunks (stats
            # ---- critical), then x8, then m0/wv, then wp ----
            x_t = []
            for t in range(CT):
                row = []
                for c in range(N // 1024):
                    xt = x_p.tile([128, 1024], bf16, tag=f"x{t}_{c}",
                                  name=f"x{t}_{c}")
                    nc.sync.dma_start(
                        xt[:], xbf[t * 128:(t + 1) * 128,
                                   c * 1024:(c + 1) * 1024])
                    row.append(xt)
                x_t.append(row)

            def xsl(ci, start, size):
                c, off = divmod(start, 1024)
                assert off + size <= 1024
                return x_t[ci][c][:, off:off + size]

            x8_t = []
            for g in range(NG):
                xt8 = x8_p.tile([128, 2, N], f8, tag=f"x8_{g}",
                                name=f"x8_{g}")
                nc.sync.dma_start(xt8[:], x8d[g][:])
                x8_t.append(xt8)
            wts_t = []
            for t in range(CT):
                wt = w_p.tile([128, 2 * C], bf16, tag=f"wts{t}")
                nc.sync.dma_start(wt[:], wts2[t * 128:(t + 1) * 128, :])
                wts_t.append(wt)
            m0_t = [wts_t[t][:, 0:C] for t in range(CT)]
            wv_t = [wts_t[t][:, C:2 * C] for t in range(CT)]
            wp_t = []
            for t in range(CT):
                wt = w_p.tile([128, C], bf16, tag=f"wp{t}")
                nc.sync.dma_start(wt[:], wpd[t * 128:(t + 1) * 128, :])
                wp_t.append(wt)

            # ---- tiny consts on the GpSimd ring (no bandwidth impact) ----
            c4_t = []
            for t in range(CT):
                c4 = cpool.tile([128, 4], f32, tag=f"c4_{t}")
                nc.gpsimd.dma_start(c4[:], consts4[t * 128:(t + 1) * 128, :])
                c4_t.append(c4)
            gam_t = [c4_t[t][:, 0:1] for t in range(CT)]
            bet_t = [c4_t[t][:, 1:2] for t in range(CT)]
            bp_t = [c4_t[t][:, 2:3] for t in range(CT)]
            qkbc_t = [c4_t[t][:, 3:4] for t in range(CT)]
            sel_t = cpool.tile([128, 8], f32, tag="sel")
            nc.gpsimd.dma_start(sel_t[:], sel[:])
            selT_t = cpool.tile([8, 128], f32, tag="selT")
            nc.gpsimd.dma_start(selT_t[:], selT[:])
            ones8_t = cpool.tile([128, 2, 128], f8, tag="ones8")
            nc.gpsimd.dma_start(ones8_t[:], ones8[:])
            esh_t = cpool.tile([128, 1], f32, tag="esh")
            nc.vector.memset(esh_t[:], ESHIFT)
            cfrac_t = cpool.tile([128, 1], f32, tag="cfrac")
            nc.vector.memset(cfrac_t[:], NDVE / 8.0)
            cinvN_t = cpool.tile([128, 1], f32, tag="cinvN")
            nc.vector.memset(cinvN_t[:], 1.0 / N)

            den_acc = acc_p.tile([1, QS], f32, tag="den")
            recip = acc_p.tile([1, QS], f32, tag="recip")
            acc_t = [acc_p.tile([128, QS], f32, tag=f"acc{t}",
                                name=f"acc{t}") for t in range(CT)]

            # ---- GroupNorm stats: DVE bn_stats (groups 0..NDVE-1) in
            # ---- parallel with ACT Identity/Square accum (the rest)
            with tc.tile_pool(name="small", bufs=1) as sm_p, \
                 tc.tile_pool(name="scr", bufs=2) as scr_p, \
                 tc.tile_pool(name="stat_ps", bufs=1, space="PSUM") as stat_ps, \
                 tc.tile_pool(name="ab_ps", bufs=2, space="PSUM") as ab_ps:
                ps_st = stat_ps.tile([8, 8], f32, tag="st")
                for t in range(CT):
                    st = sm_p.tile([128, NDVE, 6], f32, tag=f"bnst{t}")
                    for g in range(NDVE):
                        nc.vector.bn_stats(st[:, g, :],
                                           xsl(t, g * 512, 512))
                    ag = sm_p.tile([128, 2], f32, tag=f"bnag{t}")
                    nc.vector.bn_aggr(ag[:], st[:])
                    nact = 8 - NDVE
                    sx = sm_p.tile([128, nact], f32, tag=f"sx{t}")
                    sq = sm_p.tile([128, nact], f32, tag=f"sq{t}")
                    for k in range(nact):
                        g = NDVE + k
                        scr = scr_p.tile([128, 512], bf16, tag="scr")
                        nc.scalar.activation(scr[:], xsl(t, g * 512, 512),
                                             AF.Identity,
                                             accum_out=sx[:, k:k + 1])
                        scr2 = scr_p.tile([128, 512], bf16, tag="scr")
                        nc.scalar.activation(scr2[:], xsl(t, g * 512, 512),
                                             AF.Square,
                                             accum_out=sq[:, k:k + 1])
                    # combine into mean over 4096 and E[x^2] over 4096 --
                    # tensor_tensor-only ops on the otherwise-idle GpSimd
                    # (Pool rejects TensorScalar) so DVE stays on
                    # bn_stats for the next tile
                    u = sm_p.tile([128, 1], f32, tag=f"u{t}")
                    if nact > 1:
                        nc.gpsimd.tensor_tensor(u[:], sx[:, 0:1],
                                                sx[:, 1:2], alu.add)
                        for k in range(2, nact):
                            nc.gpsimd.tensor_tensor(u[:], u[:],
                                                    sx[:, k:k + 1], alu.add)
                    else:
                        nc.gpsimd.tensor_copy(u[:], sx[:])
                    mean_t = sm_p.tile([128, 1], f32, tag=f"mean{t}")
                    nc.gpsimd.tensor_tensor(mean_t[:], ag[:, 0:1],
                                            cfrac_t[:], alu.mult)
                    nc.gpsimd.tensor_tensor(u[:], u[:], cinvN_t[:],
                                            alu.mult)
                    nc.gpsimd.tensor_tensor(mean_t[:], mean_t[:], u[:],
                                            alu.add)
                    v = sm_p.tile([128, 1], f32, tag=f"v{t}")
                    if nact > 1:
                        nc.gpsimd.tensor_tensor(v[:], sq[:, 0:1],
                                                sq[:, 1:2], alu.add)
                        for k in range(2, nact):
                            nc.gpsimd.tensor_tensor(v[:], v[:],
                                                    sq[:, k:k + 1], alu.add)
                    else:
                        nc.gpsimd.tensor_copy(v[:], sq[:])
                    s2_t = sm_p.tile([128, 1], f32, tag=f"s2{t}")
                    nc.gpsimd.tensor_tensor(s2_t[:], ag[:, 0:1], ag[:, 0:1],
                                            alu.mult)
                    nc.gpsimd.tensor_tensor(s2_t[:], s2_t[:], ag[:, 1:2],
                                            alu.add)
                    nc.gpsimd.tensor_tensor(s2_t[:], s2_t[:], cfrac_t[:],
                                            alu.mult)
                    nc.gpsimd.tensor_tensor(v[:], v[:], cinvN_t[:],
                                            alu.mult)
                    nc.gpsimd.tensor_tensor(s2_t[:], s2_t[:], v[:],
                                            alu.add)
                    nc.tensor.matmul(ps_st[:, t:t + 1], sel_t[:], mean_t[:],
                                     start=True, stop=True)
                    nc.tensor.matmul(ps_st[:, 4 + t:5 + t], sel_t[:],
                                     s2_t[:], start=True, stop=True)
                st_sb = sm_p.tile([8, 8], f32, tag="st_sb")
                nc.vector.tensor_copy(st_sb[:], ps_st[:])
                mean = sm_p.tile([8, 4], f32, tag="mean")
                nc.vector.tensor_scalar(mean[:], st_sb[:, 0:4],
                                        1.0 / GS, None, op0=alu.mult)
                msq = sm_p.tile([8, 4], f32, tag="msq")
                nc.vector.tensor_scalar(msq[:], st_sb[:, 4:8],
                                        1.0 / GS, None, op0=alu.mult)
                var = sm_p.tile([8, 4], f32, tag="var")
                nc.vector.tensor_tensor(var[:], mean[:], mean[:], alu.mult)
                nc.vector.tensor_tensor(var[:], msq[:], var[:], alu.subtract)
                nc.vector.tensor_scalar(var[:], var[:], EPS, None, op0=alu.add)
                sd = sm_p.tile([8, 4], f32, tag="sd")
                nc.scalar.activation(sd[:], var[:], AF.Sqrt)
                rstd = sm_p.tile([8, 4], f32, tag="rstd")
                nc.vector.reciprocal(rstd[:], sd[:])
                A_t, A16_t, Ai16_t, Bb_t = [], [], [], []
                for t in range(CT):
                    ps_ab = ab_ps.tile([128, 2], f32, tag="ab")
                    nc.tensor.matmul(ps_ab[:, 0:1], selT_t[:],
                                     rstd[:, t:t + 1], start=True, stop=True)
                    nc.tensor.matmul(ps_ab[:, 1:2], selT_t[:],
                                     mean[:, t:t + 1], start=True, stop=True)
                    ab = cpool.tile([128, 2], f32, tag=f"ab{t}")
                    nc.vector.tensor_copy(ab[:], ps_ab[:])
                    At = cpool.tile([128, 1], f32, tag=f"A{t}")
                    nc.vector.tensor_tensor(At[:], ab[:, 0:1], gam_t[t],
                                            alu.mult)
                    At16 = cpool.tile([128, 1], f32, tag=f"A16_{t}")
                    nc.vector.tensor_scalar(At16[:], At[:], WVS, None,
                                            op0=alu.mult)
                    Ai16 = cpool.tile([128, 1], f32, tag=f"Ai16_{t}")
                    nc.vector.tensor_scalar(Ai16[:], At[:], 1.0 / WVS, None,
                                            op0=alu.mult)
                    Bt = cpool.tile([128, 1], f32, tag=f"B{t}")
                    nc.vector.tensor_tensor(Bt[:], ab[:, 1:2], At[:], alu.mult)
                    nc.vector.tensor_tensor(Bt[:], bet_t[t], Bt[:],
                                            alu.subtract)
                    Bb = cpool.tile([128, 1], bf16, tag=f"Bb{t}")
                    nc.vector.tensor_copy(Bb[:], Bt[:])
                    A_t.append(At)
                    A16_t.append(At16)
                    Ai16_t.append(Ai16)
                    Bb_t.append(Bb)

                # fp8 DR weight tiles (prescaled x16; t = 2g + i):
                #   wv8 = fp8(A16*wv rows)   on DVE, FIRST (gates vprod)
                #   m08 = fp8(A16*m0 rows)   on ACT, concurrently
                #   wp8 = fp8(16 * wp.T)     on ACT, last (finalize only)
                wv8_t, m08_t, wp8_t = [], [], []
                for g in range(NG):
                    w8 = w_p.tile([128, 2, C], f8, tag=f"wv8_{g}")
                    for i in range(2):
                        t = 2 * g + i
                        nc.vector.tensor_scalar(w8[:, i, :], wv_t[t],
                                                A16_t[t][:], None,
                                                op0=alu.mult)
                    wv8_t.append(w8)
                for g in range(NG):
                    m8 = w_p.tile([128, 2, C], f8, tag=f"m08_{g}")
                    for i in range(2):
                        t = 2 * g + i
                        nc.scalar.activation(m8[:, i, :], m0_t[t],
                                             AF.Identity, scale=A16_t[t][:])
                    m08_t.append(m8)
                for g in range(NG):
                    p8 = w_p.tile([128, 2, C], f8, tag=f"wp8_{g}")
                    for i in range(2):
                        t = 2 * g + i
                        nc.scalar.activation(p8[:, i, :], wp_t[t][:],
                                             AF.Identity, scale=WVS * WVS)
                    wp8_t.append(p8)

            # ---- post-stats region: one PSUM layout (3 + 3 + small) ----
            with tc.tile_pool(name="mm_ps", bufs=3, space="PSUM") as mm_ps, \
                 tc.tile_pool(name="att_ps", bufs=3, space="PSUM") as att_ps, \
                 tc.tile_pool(name="sm2_ps", bufs=1, space="PSUM") as smp:

                def vprod(half):
                    j0 = half * JQ
                    v8_t = []
                    for jt in range(JT):
                        ps = mm_ps.tile([128, 512], f32, tag="mm")
                        for g in range(NG):
                            mm8(ps[:],
                                x8_t[g][:, :, j0 + jt * 128:
                                        j0 + (jt + 1) * 128],
                                wv8_t[g][:], g == 0, g == NG - 1)
                        kt2, slot = divmod(jt, 2)
                        if slot == 0:
                            vt = v8_p.tile([128, 2, C], f8, tag="v8")
                            v8_t.append(vt)
                        if jt % 4 < 2:
                            nc.scalar.copy(v8_t[kt2][:, slot, :], ps[:])
                        else:
                            nc.vector.tensor_copy(v8_t[kt2][:, slot, :],
                                                  ps[:])
                    return v8_t

                # half-0 V first: only needs wv8 + x8; warms the PE while
                # the bias/q chain resolves
                v8_half0 = vprod(0)

                # bias terms from RAW weights:
                #   qkb = M0@B + wk^T bq (host const);  Abias = A*qkb
                #   tv  = wv@B  (for the projection-bias fold)
                abias_t, tvb_t = [], []
                for co in range(CT):
                    ps_b = smp.tile([128, 2], f32, tag="bb")
                    for ci in range(CT):
                        mm(ps_b[:, 0:1],
                           m0_t[ci][:, co * 128:(co + 1) * 128], Bb_t[ci][:],
                           ci == 0, ci == CT - 1)
                    for ci in range(CT):
                        mm(ps_b[:, 1:2],
                           wv_t[ci][:, co * 128:(co + 1) * 128], Bb_t[ci][:],
                           ci == 0, ci == CT - 1)
                    ab2 = cpool.tile([128, 1], f32, tag=f"abias{co}")
                    nc.vector.tensor_tensor(ab2[:], ps_b[:, 0:1],
                                            qkbc_t[co], alu.add)
                    nc.vector.tensor_tensor(ab2[:], ab2[:], A_t[co][:],
                                            alu.mult)
                    abias_t.append(ab2)
                    tvb = cpool.tile([128, 1], bf16, tag=f"tvb{co}")
                    nc.vector.tensor_copy(tvb[:], ps_b[:, 1:2])
                    tvb_t.append(tvb)

                # qk projection (fp8 DR) -> fp8 DR tiles q8[g][:, i, :]
                # psum carries 16*q (m08 prescale); scale back with A/16
                q8_t = [q8_p.tile([128, 2, QS], f8, tag=f"q8_{g}",
                                  name=f"q8_{g}") for g in range(NG)]
                for co in range(CT):
                    g, i = divmod(co, 2)
                    for nn in range(QS // 512):
                        ps = mm_ps.tile([128, 512], f32, tag="mm")
                        for gi in range(NG):
                            mm8(ps[:],
                                m08_t[gi][:, :, co * 128:(co + 1) * 128],
                                x8_t[gi][:, :, nn * 512:(nn + 1) * 512],
                                gi == 0, gi == NG - 1)
                        nc.vector.tensor_scalar(
                            q8_t[g][:, i, nn * 512:(nn + 1) * 512],
                            ps[:], Ai16_t[co][:], abias_t[co][:],
                            op0=alu.mult, op1=alu.add)

                # device projection bias bpd = 16*(wp/16)@tv + bp_eff, and
                # xb = x_residual + bpd so the finalize needs one STT
                bpd_t, xb_t = [], []
                for co in range(CT):
                    ps_u = smp.tile([128, 1], f32, tag="u")
                    for ci in range(CT):
                        mm(ps_u[:], wp_t[ci][:, co * 128:(co + 1) * 128],
                           tvb_t[ci][:], ci == 0, ci == CT - 1)
                    bpd = f_p.tile([128, 1], f32, tag=f"bpd{co}")
                    nc.vector.scalar_tensor_tensor(
                        bpd[:], ps_u[:], WVS, bp_t[co],
                        op0=alu.mult, op1=alu.add)
                    bpd_t.append(bpd)
                    xb = f_p.tile([128, QS], bf16, tag=f"xb{co}")
                    nc.vector.tensor_scalar(xb[:], x_t[co][0][:],
                                            bpd[:], None, op0=alu.add)
                    xb_t.append(xb)

                # ---- attention over key halves (fp8 DoubleRow) ----
                o_p = outer.enter_context(tc.tile_pool(name="outp", bufs=3))
                rb = f_p.tile([128, QS], f32, tag="rb")
                accn8_t = [f_p.tile([128, 2, QS], f8, tag=f"accn8_{g}",
                                    name=f"accn8_{g}") for g in range(NG)]
                for half in range(NHALF):
                    j0 = half * JQ
                    v8_t = v8_half0 if half == 0 else vprod(1)

                    # scores + exp for BOTH chunks first, so the last
                    # chunk's exps (ACT) overlap the first chunk's attnV
                    e8_c = []
                    for ch in range(NCH):
                        i0 = ch * ICH
                        e8_t = []
                        for jt in range(JT):
                            ps = mm_ps.tile([128, ICH], f32, tag="mm")
                            for g in range(NG):
                                mm8(ps[:],
                                    x8_t[g][:, :, j0 + jt * 128:
                                            j0 + (jt + 1) * 128],
                                    q8_t[g][:, :, i0:i0 + ICH],
                                    g == 0, g == NG - 1)
                            kt2, slot = divmod(jt, 2)
                            if slot == 0:
                                et = e8_p.tile([128, 2, ICH], f8, tag="e8")
                                e8_t.append(et)
                            nc.scalar.activation(e8_t[kt2][:, slot, :],
                                                 ps[:], AF.Exp, scale=SCALE,
                                                 bias=esh_t[:])
                        e8_c.append(e8_t)

                    for ch in range(NCH):
                        i0 = ch * ICH
                        e8_t = e8_c[ch]
                        # denominator: all-ones stationary (every output
                        # partition carries the same key-sum; row 0 used)
                        ps_d = att_ps.tile([128, ICH], f32, tag="att")
                        for kt2 in range(KT2):
                            mm8(ps_d[:], ones8_t[:], e8_t[kt2][:],
                                kt2 == 0, kt2 == KT2 - 1)
                        if half == 0:
                            nc.vector.tensor_copy(den_acc[:, i0:i0 + ICH],
                                                  ps_d[0:1, :])
                        else:
                            nc.vector.tensor_tensor(den_acc[:, i0:i0 + ICH],
                                                    den_acc[:, i0:i0 + ICH],
                                                    ps_d[0:1, :], alu.add)
                            nc.vector.reciprocal(recip[:, i0:i0 + ICH],
                                                 den_acc[:, i0:i0 + ICH])
                            # broadcast early: depends only on the denom
                            nc.gpsimd.partition_broadcast(
                                rb[:, i0:i0 + ICH], recip[:, i0:i0 + ICH])
                        for co in range(CT):
                            ps_a = att_ps.tile([128, ICH], f32, tag="att")
                            for kt2 in range(KT2):
                                mm8(ps_a[:],
                                    v8_t[kt2][:, :, co * 128:(co + 1) * 128],
                                    e8_t[kt2][:], kt2 == 0, kt2 == KT2 - 1)
                            sl = slice(i0, i0 + ICH)
                            if half == 0:
                                nc.vector.tensor_copy(
                                    acc_t[co][:, i0:i0 + ICH], ps_a[:])
                            else:
                                nc.vector.tensor_tensor(
                                    acc_t[co][:, i0:i0 + ICH],
                                    acc_t[co][:, i0:i0 + ICH], ps_a[:],
                                    alu.add)
                                # normalize into the fp8 DR tile right
                                # away so the projection is only one
                                # DVE op behind the last attnV chain
                                g, i = divmod(co, 2)
                                nc.vector.tensor_tensor(
                                    accn8_t[g][:, i, sl],
                                    acc_t[co][:, sl], rb[:, sl], alu.mult)
                        if half == 0:
                            continue
                        # finalize this chunk right away (overlaps the
                        # next chunk's attnV): accn8 = fp8(16*attnout),
                        # proj = DR(wp8, accn8) = 256*out,
                        # out = ps/256 + (x + bpd)
                        for co in range(CT):
                            ps = att_ps.tile([128, 512], f32, tag="att")
                            for g in range(NG):
                                mm8(ps[:],
                                    wp8_t[g][:, :, co * 128:(co + 1) * 128],
                                    accn8_t[g][:, :, sl],
                                    g == 0, g == NG - 1)
                            ot = o_p.tile([128, 512], f32, tag="o")
                            nc.vector.scalar_tensor_tensor(
                                ot[:], ps[:], 1.0 / (WVS * WVS),
                                xb_t[co][:, sl],
                                op0=alu.mult, op1=alu.add)
                            nc.sync.dma_start(
                                out_d[co * 128:(co + 1) * 128, sl], ot[:])

    nc.compile()
    return nc


def kernel(x, gn_gamma, gn_beta, wq, bq, wk, bk, wv, bv, wp, bp):
    import ml_dtypes
    from concourse import bass_utils

    if "nc" not in _CACHE:
        _CACHE["nc"] = _build()
    nc = _CACHE["nc"]

    x = np.asarray(x, np.float32)
    f = np.float32
    bf = ml_dtypes.bfloat16
    f8 = ml_dtypes.float8_e4m3
    wq32 = np.asarray(wq, f)
    wk32 = np.asarray(wk, f)
    m0T = (wq32.T @ wk32).astype(bf)  # (wk^T wq)^T
    qkbc = (wk32.T @ np.asarray(bq, f)).reshape(C, 1)
    wvT = np.asarray(wv, f).T.astype(bf)
    wts2 = np.ascontiguousarray(np.concatenate([m0T, wvT], axis=1))
    wpd = np.ascontiguousarray((np.asarray(wp, f).T / WVS).astype(bf))
    bp_eff = (np.asarray(bp, f)
              + np.asarray(wp, f) @ np.asarray(bv, f)).reshape(C, 1)
    consts4 = np.ascontiguousarray(np.concatenate([
        np.asarray(gn_gamma, f).reshape(C, 1),
        np.asarray(gn_beta, f).reshape(C, 1),
        bp_eff, qkbc], axis=1))
    sel = np.zeros((128, 8), f)
    for p in range(128):
        sel[p, p // GS] = 1.0
    common = {
        "wts2": wts2, "wpd": wpd, "consts4": consts4,
        "sel": sel, "selT": np.ascontiguousarray(sel.T),
        "ones8": np.ones((128, 2, 128), f8),
    }
    in_maps = []
    for core in range(NCORES):
        b, s = divmod(core, 4)
        xb = x[b].reshape(C, N)
        # roll so this core's query block occupies columns 0:QS; key order
        # is permuted identically for all key-side tensors, and softmax
        # sums are order-invariant, so the program is core-independent.
        xperm = np.ascontiguousarray(np.roll(xb, -s * QS, axis=1))
        im = {**common, "xbf": xperm.astype(bf)}
        for g in range(NG):
            x8g = xperm[g * 256:(g + 1) * 256].reshape(2, 128, N)
            im[f"x8_{g}"] = np.ascontiguousarray(
                x8g.transpose(1, 0, 2)).astype(f8)
        in_maps.append(im)

    res = bass_utils.run_bass_kernel_spmd(nc, in_maps,
                                          core_ids=list(range(NCORES)))
    _CACHE["last_result"] = res

    out = np.empty((B, C, N), np.float32)
    for core in range(NCORES):
        b, s = divmod(core, 4)
        out[b][:, s * QS:(s + 1) * QS] = res.results[core]["out"]
    return out.reshape(B, C, H, W)


# revision 33
# speedup vs baseline: 1.0421x; 1.0421x over previous
"""AttnBlock v8: fp8 DoubleRow everywhere, restructured head/tail.

Sharding: core = (batch b in {0,1}) x (query slice s in {0..3}, 1024
queries).  Each core redundantly computes full V^T for its batch
(avoids cross-core collectives), attention for its query slice only.
The host rolls x columns per core so the core's query block is always
columns 0:1024 -- identical program, per-core data.

Math: h = GN(x) = A_c * x + B_c per channel (A, B from runtime stats).
  q = (wq*A)@x + (wq@B + bq)     weight columns scaled on device
  k = (wk*A)@x   (k-bias dropped: per-query-constant under softmax)
  v = (wv*A)@x + const; v-bias folded into the projection bias:
      bp_dev = bp + wp@bv + wp@(wv@B).

v8 over v7 (224 -> 158 -> 153 -> this):
  - half-0 v-production is hoisted before the bias folds / q
    projection: it only depends on wv8 + x8, so the PE warms up (HAM)
    and works while the bias chain resolves on DVE.
  - one PSUM layout for the whole post-stats region (mm 3 + att 3 +
    small 2 = 8 banks); the denominator and the final projection share
    the att pool, so no pool-boundary serialization.
  - v8 psum->fp8 copies split ACT/DVE again (an all-DVE drain at
    ~810ns/tile throttled the v-production below PE pace).
  - acc tiles bf16 (cheaper DVE accumulate + normalize reads).
"""

import os
import sys

import numpy as np

for _p in ("/opt/trn_rl_repo", "/root/.axon_site/_ro/trn_rl_repo"):
    if os.path.isdir(_p) and _p not in sys.path:
        sys.path.insert(0, _p)

B, C, H, W = 2, 512, 64, 64
N = H * W
G = 32
GS = C // G
EPS = 1e-6
NCORES = 8
QS = N // 4               # 1024 queries per core
NHALF = 2                 # key halves
JQ = N // NHALF           # 2048 keys per half
JT = JQ // 128            # 16 key tiles per half
KT2 = JT // 2             # 8 DoubleRow key groups per half
ICH = 512                 # query chunk
NCH = QS // ICH           # 2 chunks
CT = C // 128             # 4 channel tiles
NG = 2                    # DoubleRow channel groups (256 ch each)
SCALE = float(C) ** -0.5
WVS = 16.0                # wv / m0 / wp prescale into fp8
ESHIFT = -3.0             # exp(s + ESHIFT): keep e in fp8e4m3 range
                          # (max scaled score ~7.2; e4m3 max normal 240);
                          # a constant shift cancels in the softmax ratio
NDVE = 6                  # of 8 per-tile stat groups on DVE (rest ACT)

_CACHE = {}


def _build():
    import contextlib

    import concourse.mybir as mybir
    import concourse.tile as tile
    from concourse import bacc
    from concourse.alu_op_type import AluOpType as alu

    f32 = mybir.dt.float32
    bf16 = mybir.dt.bfloat16
    f8 = mybir.dt.float8e4
    AF = mybir.ActivationFunctionType
    PM = mybir.MatmulPerfMode

    nc = bacc.Bacc("TRN2", target_bir_lowering=False, debug=False,
                   num_devices=NCORES)

    xqd = nc.dram_tensor("xq", [C, QS], bf16, kind="ExternalInput").ap()
    x8d = [nc.dram_tensor(f"x8_{g}", [128, 2, N], f8,
                          kind="ExternalInput").ap() for g in range(NG)]
    # wts2 = [m0T | wvT]; wpd = wp.T/16; consts4 = [gamma | beta |
    # bp_eff | qkbc] as columns
    wts2 = nc.dram_tensor("wts2", [C, 2 * C], bf16, kind="ExternalInput").ap()
    wpd = nc.dram_tensor("wpd", [C, C], bf16, kind="ExternalInput").ap()
    consts4 = nc.dram_tensor("consts4", [C, 4], f32,
                             kind="ExternalInput").ap()
    sel = nc.dram_tensor("sel", [128, 8], f32, kind="ExternalInput").ap()
    selT = nc.dram_tensor("selT", [8, 128], f32, kind="ExternalInput").ap()
    ones8 = nc.dram_tensor("ones8", [128, 2, 128], f8,
                           kind="ExternalInput").ap()
    out_d = nc.dram_tensor("out", [C, QS], f32, kind="ExternalOutput").ap()

    def mm(ps, lhsT, rhs, start, stop):
        nc.tensor.matmul(ps, lhsT, rhs, start=start, stop=stop)

    def mm8(ps, lhsT, rhs, start, stop):
        nc.tensor.matmul(ps, lhsT, rhs, start=start, stop=stop,
                         perf_mode=PM.DoubleRow)

    with tile.TileContext(nc) as tc:
        outer = contextlib.ExitStack()
        with outer:
            cpool = outer.enter_context(tc.tile_pool(name="const", bufs=1))
            x_p = outer.enter_context(tc.tile_pool(name="xbf", bufs=1))
            x8_p = outer.enter_context(tc.tile_pool(name="x8", bufs=1))
            acc_p = outer.enter_context(tc.tile_pool(name="acc", bufs=1))
            w_p = outer.enter_context(tc.tile_pool(name="wts", bufs=1))
            q8_p = outer.enter_context(tc.tile_pool(name="q8", bufs=1))
            v8_p = outer.enter_context(tc.tile_pool(name="v8", bufs=2 * KT2))
            e8_p = outer.enter_context(tc.tile_pool(name="e8",
                                                    bufs=2 * KT2 + 2))
            f_p = outer.enter_context(tc.tile_pool(name="fin", bufs=1))

            # ---- Sync ring, in transfer-priority order: x8 in column
            # ---- chunks (stats run on the fp8 x directly), then the
            # ---- bf16 query-slice (residual), then m0/wv, then wp ----
            x8_t = []
            for g in range(NG):
                xt8 = x8_p.tile([128, 2, N], f8, tag=f"x8_{g}",
                                name=f"x8_{g}")
                for c in range(N // 1024):
                    nc.sync.dma_start(
                        xt8[:, :, c * 1024:(c + 1) * 1024],
                        x8d[g][:, :, c * 1024:(c + 1) * 1024])
                x8_t.append(xt8)

            def xsl(ci, start, size):
                # stats input: channel tile ci lives in x8 group ci//2,
                # pair-slot ci%2 (channel = 256*(ci//2) + 128*(ci%2) + p)
                return x8_t[ci // 2][:, ci % 2, start:start + size]

            xq_t = []
            for t in range(CT):
                xt = x_p.tile([128, QS], bf16, tag=f"xq{t}",
                              name=f"xq{t}")
                nc.sync.dma_start(xt[:], xqd[t * 128:(t + 1) * 128, :])
                xq_t.append(xt)
            wts_t = []
            for t in range(CT):
                wt = w_p.tile([128, 2 * C], bf16, tag=f"wts{t}")
                nc.sync.dma_start(wt[:], wts2[t * 128:(t + 1) * 128, :])
                wts_t.append(wt)
            m0_t = [wts_t[t][:, 0:C] for t in range(CT)]
            wv_t = [wts_t[t][:, C:2 * C] for t in range(CT)]
            wp_t = []
            for t in range(CT):
                wt = w_p.tile([128, C], bf16, tag=f"wp{t}")
                nc.sync.dma_start(wt[:], wpd[t * 128:(t + 1) * 128, :])
                wp_t.append(wt)

            # ---- tiny consts on the GpSimd ring (no bandwidth impact) ----
            c4_t = []
            for t in range(CT):
                c4 = cpool.tile([128, 4], f32, tag=f"c4_{t}")
                nc.gpsimd.dma_start(c4[:], consts4[t * 128:(t + 1) * 128, :])
                c4_t.append(c4)
            gam_t = [c4_t[t][:, 0:1] for t in range(CT)]
            bet_t = [c4_t[t][:, 1:2] for t in range(CT)]
            bp_t = [c4_t[t][:, 2:3] for t in range(CT)]
            qkbc_t = [c4_t[t][:, 3:4] for t in range(CT)]
            sel_t = cpool.tile([128, 8], f32, tag="sel")
            nc.gpsimd.dma_start(sel_t[:], sel[:])
            selT_t = cpool.tile([8, 128], f32, tag="selT")
            nc.gpsimd.dma_start(selT_t[:], selT[:])
            ones8_t = cpool.tile([128, 2, 128], f8, tag="ones8")
            nc.gpsimd.dma_start(ones8_t[:], ones8[:])
            esh_t = cpool.tile([128, 1], f32, tag="esh")
            nc.vector.memset(esh_t[:], ESHIFT)
            cfrac_t = cpool.tile([128, 1], f32, tag="cfrac")
            nc.vector.memset(cfrac_t[:], NDVE / 8.0)
            cinvN_t = cpool.tile([128, 1], f32, tag="cinvN")
            nc.vector.memset(cinvN_t[:], 1.0 / N)

            den_acc = acc_p.tile([1, QS], f32, tag="den")
            recip = acc_p.tile([1, QS], f32, tag="recip")
            acc_t = [acc_p.tile([128, QS], f32, tag=f"acc{t}",
                                name=f"acc{t}") for t in range(CT)]

            # ---- GroupNorm stats: DVE bn_stats (groups 0..NDVE-1) in
            # ---- parallel with ACT Identity/Square accum (the rest)
            with tc.tile_pool(name="small", bufs=1) as sm_p, \
                 tc.tile_pool(name="scr", bufs=2) as scr_p, \
                 tc.tile_pool(name="stat_ps", bufs=1, space="PSUM") as stat_ps, \
                 tc.tile_pool(name="ab_ps", bufs=2, space="PSUM") as ab_ps:
                ps_st = stat_ps.tile([8, 8], f32, tag="st")
                for t in range(CT):
                    st = sm_p.tile([128, NDVE, 6], f32, tag=f"bnst{t}")
                    for g in range(NDVE):
                        nc.vector.bn_stats(st[:, g, :],
                                           xsl(t, g * 512, 512))
                    ag = sm_p.tile([128, 2], f32, tag=f"bnag{t}")
                    nc.vector.bn_aggr(ag[:], st[:])
                    nact = 8 - NDVE
                    sx = sm_p.tile([128, nact], f32, tag=f"sx{t}")
                    sq = sm_p.tile([128, nact], f32, tag=f"sq{t}")
                    for k in range(nact):
                        g = NDVE + k
                        scr = scr_p.tile([128, 512], bf16, tag="scr")
                        nc.scalar.activation(scr[:], xsl(t, g * 512, 512),
                                             AF.Identity,
                                             accum_out=sx[:, k:k + 1])
                        scr2 = scr_p.tile([128, 512], bf16, tag="scr")
                        nc.scalar.activation(scr2[:], xsl(t, g * 512, 512),
                                             AF.Square,
                                             accum_out=sq[:, k:k + 1])
                    # combine into mean over 4096 and E[x^2] over 4096 --
                    # tensor_tensor-only ops on the otherwise-idle GpSimd
                    # (Pool rejects TensorScalar) so DVE stays on
                    # bn_stats for the next tile
                    u = sm_p.tile([128, 1], f32, tag=f"u{t}")
                    if nact > 1:
                        nc.gpsimd.tensor_tensor(u[:], sx[:, 0:1],
                                                sx[:, 1:2], alu.add)
                        for k in range(2, nact):
                            nc.gpsimd.tensor_tensor(u[:], u[:],
                                                    sx[:, k:k + 1], alu.add)
                    else:
                        nc.gpsimd.tensor_copy(u[:], sx[:])
                    mean_t = sm_p.tile([128, 1], f32, tag=f"mean{t}")
                    nc.gpsimd.tensor_tensor(mean_t[:], ag[:, 0:1],
                                            cfrac_t[:], alu.mult)
                    nc.gpsimd.tensor_tensor(u[:], u[:], cinvN_t[:],
                                            alu.mult)
                    nc.gpsimd.tensor_tensor(mean_t[:], mean_t[:], u[:],
                                            alu.add)
                    v = sm_p.tile([128, 1], f32, tag=f"v{t}")
                    if nact > 1:
                        nc.gpsimd.tensor_tensor(v[:], sq[:, 0:1],
                                                sq[:, 1:2], alu.add)
                        for k in range(2, nact):
                            nc.gpsimd.tensor_tensor(v[:], v[:],
                                                    sq[:, k:k + 1], alu.add)
                    else:
                        nc.gpsimd.tensor_copy(v[:], sq[:])
                    s2_t = sm_p.tile([128, 1], f32, tag=f"s2{t}")
                    nc.gpsimd.tensor_tensor(s2_t[:], ag[:, 0:1], ag[:, 0:1],
                                            alu.mult)
                    nc.gpsimd.tensor_tensor(s2_t[:], s2_t[:], ag[:, 1:2],
                                            alu.add)
                    nc.gpsimd.tensor_tensor(s2_t[:], s2_t[:], cfrac_t[:],
                                            alu.mult)
                    nc.gpsimd.tensor_tensor(v[:], v[:], cinvN_t[:],
                                            alu.mult)
                    nc.gpsimd.tensor_tensor(s2_t[:], s2_t[:], v[:],
                                            alu.add)
                    nc.tensor.matmul(ps_st[:, t:t + 1], sel_t[:], mean_t[:],
                                     start=True, stop=True)
                    nc.tensor.matmul(ps_st[:, 4 + t:5 + t], sel_t[:],
                                     s2_t[:], start=True, stop=True)
                st_sb = sm_p.tile([8, 8], f32, tag="st_sb")
                nc.vector.tensor_copy(st_sb[:], ps_st[:])
                mean = sm_p.tile([8, 4], f32, tag="mean")
                nc.vector.tensor_scalar(mean[:], st_sb[:, 0:4],
                                        1.0 / GS, None, op0=alu.mult)
                msq = sm_p.tile([8, 4], f32, tag="msq")
                nc.vector.tensor_scalar(msq[:], st_sb[:, 4:8],
                                        1.0 / GS, None, op0=alu.mult)
                var = sm_p.tile([8, 4], f32, tag="var")
                nc.vector.tensor_tensor(var[:], mean[:], mean[:], alu.mult)
                nc.vector.tensor_tensor(var[:], msq[:], var[:], alu.subtract)
                nc.vector.tensor_scalar(var[:], var[:], EPS, None, op0=alu.add)
                sd = sm_p.tile([8, 4], f32, tag="sd")
                nc.scalar.activation(sd[:], var[:], AF.Sqrt)
                rstd = sm_p.tile([8, 4], f32, tag="rstd")
                nc.vector.reciprocal(rstd[:], sd[:])
                A_t, A16_t, Ai16_t, Bb_t = [], [], [], []
                for t in range(CT):
                    ps_ab = ab_ps.tile([128, 2], f32, tag="ab")
                    nc.tensor.matmul(ps_ab[:, 0:1], selT_t[:],
                                     rstd[:, t:t + 1], start=True, stop=True)
                    nc.tensor.matmul(ps_ab[:, 1:2], selT_t[:],
                                     mean[:, t:t + 1], start=True, stop=True)
                    ab = cpool.tile([128, 2], f32, tag=f"ab{t}")
                    nc.vector.tensor_copy(ab[:], ps_ab[:])
                    At = cpool.tile([128, 1], f32, tag=f"A{t}")
                    nc.vector.tensor_tensor(At[:], ab[:, 0:1], gam_t[t],
                                            alu.mult)
                    At16 = cpool.tile([128, 1], f32, tag=f"A16_{t}")
                    nc.vector.tensor_scalar(At16[:], At[:], WVS, None,
                                            op0=alu.mult)
                    Ai16 = cpool.tile([128, 1], f32, tag=f"Ai16_{t}")
                    nc.vector.tensor_scalar(Ai16[:], At[:], 1.0 / WVS, None,
                                            op0=alu.mult)
                    Bt = cpool.tile([128, 1], f32, tag=f"B{t}")
                    nc.vector.tensor_tensor(Bt[:], ab[:, 1:2], At[:], alu.mult)
                    nc.vector.tensor_tensor(Bt[:], bet_t[t], Bt[:],
                                            alu.subtract)
                    Bb = cpool.tile([128, 1], bf16, tag=f"Bb{t}")
                    nc.vector.tensor_copy(Bb[:], Bt[:])
                    A_t.append(At)
                    A16_t.append(At16)
                    Ai16_t.append(Ai16)
                    Bb_t.append(Bb)

                # fp8 DR weight tiles (prescaled x16; t = 2g + i), on DVE:
                #   wv8 = fp8(A16*wv rows), m08 = fp8(A16*m0 rows),
                #   wp8 = fp8(16 * wp.T) [host sent wp.T/16 -> scale 256]
                wv8_t, m08_t, wp8_t = [], [], []
                for g in range(NG):
                    w8 = w_p.tile([128, 2, C], f8, tag=f"wv8_{g}")
                    m8 = w_p.tile([128, 2, C], f8, tag=f"m08_{g}")
                    p8 = w_p.tile([128, 2, C], f8, tag=f"wp8_{g}")
                    for i in range(2):
                        t = 2 * g + i
                        nc.vector.tensor_scalar(w8[:, i, :], wv_t[t],
                                                A16_t[t][:], None,
                                                op0=alu.mult)
                        nc.vector.tensor_scalar(m8[:, i, :], m0_t[t],
                                                A16_t[t][:], None,
                                                op0=alu.mult)
                        nc.vector.tensor_scalar(p8[:, i, :], wp_t[t][:],
                                                WVS * WVS, None,
                                                op0=alu.mult)
                    wv8_t.append(w8)
                    m08_t.append(m8)
                    wp8_t.append(p8)

            # ---- post-stats region: one PSUM layout (3 + 3 + small) ----
            with tc.tile_pool(name="mm_ps", bufs=3, space="PSUM") as mm_ps, \
                 tc.tile_pool(name="att_ps", bufs=3, space="PSUM") as att_ps, \
                 tc.tile_pool(name="sm2_ps", bufs=1, space="PSUM") as smp:

                def vprod(half):
                    j0 = half * JQ
                    v8_t = []
                    for jt in range(JT):
                        ps = mm_ps.tile([128, 512], f32, tag="mm")
                        for g in range(NG):
                            mm8(ps[:],
                                x8_t[g][:, :, j0 + jt * 128:
                                        j0 + (jt + 1) * 128],
                                wv8_t[g][:], g == 0, g == NG - 1)
                        kt2, slot = divmod(jt, 2)
                        if slot == 0:
                            vt = v8_p.tile([128, 2, C], f8, tag="v8")
                            v8_t.append(vt)
                        if jt % 4 < 2:
                            nc.scalar.copy(v8_t[kt2][:, slot, :], ps[:])
                        else:
                            nc.vector.tensor_copy(v8_t[kt2][:, slot, :],
                                                  ps[:])
                    return v8_t

                # half-0 V first: only needs wv8 + x8; warms the PE while
                # the bias/q chain resolves
                v8_half0 = vprod(0)

                # bias terms from RAW weights:
                #   qkb = M0@B + wk^T bq (host const);  Abias = A*qkb
                #   tv  = wv@B  (for the projection-bias fold)
                abias_t, tvb_t = [], []
                for co in range(CT):
                    ps_b = smp.tile([128, 2], f32, tag="bb")
                    for ci in range(CT):
                        mm(ps_b[:, 0:1],
                           m0_t[ci][:, co * 128:(co + 1) * 128], Bb_t[ci][:],
                           ci == 0, ci == CT - 1)
                    for ci in range(CT):
                        mm(ps_b[:, 1:2],
                           wv_t[ci][:, co * 128:(co + 1) * 128], Bb_t[ci][:],
                           ci == 0, ci == CT - 1)
                    ab2 = cpool.tile([128, 1], f32, tag=f"abias{co}")
                    nc.vector.tensor_tensor(ab2[:], ps_b[:, 0:1],
                                            qkbc_t[co], alu.add)
                    nc.vector.tensor_tensor(ab2[:], ab2[:], A_t[co][:],
                                            alu.mult)
                    abias_t.append(ab2)
                    tvb = cpool.tile([128, 1], bf16, tag=f"tvb{co}")
                    nc.vector.tensor_copy(tvb[:], ps_b[:, 1:2])
                    tvb_t.append(tvb)

                # qk projection (fp8 DR) -> fp8 DR tiles q8[g][:, i, :]
                # psum carries 16*q (m08 prescale); scale back with A/16
                q8_t = [q8_p.tile([128, 2, QS], f8, tag=f"q8_{g}",
                                  name=f"q8_{g}") for g in range(NG)]
                for co in range(CT):
                    g, i = divmod(co, 2)
                    for nn in range(QS // 512):
                        ps = mm_ps.tile([128, 512], f32, tag="mm")
                        for gi in range(NG):
                            mm8(ps[:],
                                m08_t[gi][:, :, co * 128:(co + 1) * 128],
                                x8_t[gi][:, :, nn * 512:(nn + 1) * 512],
                                gi == 0, gi == NG - 1)
                        nc.vector.tensor_scalar(
                            q8_t[g][:, i, nn * 512:(nn + 1) * 512],
                            ps[:], Ai16_t[co][:], abias_t[co][:],
                            op0=alu.mult, op1=alu.add)

                # device projection bias bpd = 16*(wp/16)@tv + bp_eff, and
                # xb = x_residual + bpd so the finalize needs one STT
                bpd_t, xb_t = [], []
                for co in range(CT):
                    ps_u = smp.tile([128, 1], f32, tag="u")
                    for ci in range(CT):
                        mm(ps_u[:], wp_t[ci][:, co * 128:(co + 1) * 128],
                           tvb_t[ci][:], ci == 0, ci == CT - 1)
                    bpd = f_p.tile([128, 1], f32, tag=f"bpd{co}")
                    nc.vector.scalar_tensor_tensor(
                        bpd[:], ps_u[:], WVS, bp_t[co],
                        op0=alu.mult, op1=alu.add)
                    bpd_t.append(bpd)
                    xb = f_p.tile([128, QS], bf16, tag=f"xb{co}")
                    nc.vector.tensor_scalar(xb[:], xq_t[co][:],
                                            bpd[:], None, op0=alu.add)
                    xb_t.append(xb)

                # ---- attention over key halves (fp8 DoubleRow) ----
                o_p = outer.enter_context(tc.tile_pool(name="outp", bufs=3))
                rb = f_p.tile([128, QS], f32, tag="rb")
                accn8_t = [f_p.tile([128, 2, QS], f8, tag=f"accn8_{g}",
                                    name=f"accn8_{g}") for g in range(NG)]
                for half in range(NHALF):
                    j0 = half * JQ
                    v8_t = v8_half0 if half == 0 else vprod(1)

                    # scores + exp for BOTH chunks first, so the last
                    # chunk's exps (ACT) overlap the first chunk's attnV
                    e8_c = []
                    for ch in range(NCH):
                        i0 = ch * ICH
                        e8_t = []
                        for jt in range(JT):
                            ps = mm_ps.tile([128, ICH], f32, tag="mm")
                            for g in range(NG):
                                mm8(ps[:],
                                    x8_t[g][:, :, j0 + jt * 128:
                                            j0 + (jt + 1) * 128],
                                    q8_t[g][:, :, i0:i0 + ICH],
                                    g == 0, g == NG - 1)
                            kt2, slot = divmod(jt, 2)
                            if slot == 0:
                                et = e8_p.tile([128, 2, ICH], f8, tag="e8")
                                e8_t.append(et)
                            nc.scalar.activation(e8_t[kt2][:, slot, :],
                                                 ps[:], AF.Exp, scale=SCALE,
                                                 bias=esh_t[:])
                        e8_c.append(e8_t)

                    for ch in range(NCH):
                        i0 = ch * ICH
                        e8_t = e8_c[ch]
                        # denominator: all-ones stationary (every output
                        # partition carries the same key-sum; row 0 used)
                        ps_d = att_ps.tile([128, ICH], f32, tag="att")
                        for kt2 in range(KT2):
                            mm8(ps_d[:], ones8_t[:], e8_t[kt2][:],
                                kt2 == 0, kt2 == KT2 - 1)
                        if half == 0:
                            nc.vector.tensor_copy(den_acc[:, i0:i0 + ICH],
                                                  ps_d[0:1, :])
                        else:
                            nc.vector.tensor_tensor(den_acc[:, i0:i0 + ICH],
                                                    den_acc[:, i0:i0 + ICH],
                                                    ps_d[0:1, :], alu.add)
                            nc.vector.reciprocal(recip[:, i0:i0 + ICH],
                                                 den_acc[:, i0:i0 + ICH])
                            # broadcast early: depends only on the denom
                            nc.gpsimd.partition_broadcast(
                                rb[:, i0:i0 + ICH], recip[:, i0:i0 + ICH])
                        for co in range(CT):
                            ps_a = att_ps.tile([128, ICH], f32, tag="att")
                            for kt2 in range(KT2):
                                mm8(ps_a[:],
                                    v8_t[kt2][:, :, co * 128:(co + 1) * 128],
                                    e8_t[kt2][:], kt2 == 0, kt2 == KT2 - 1)
                            sl = slice(i0, i0 + ICH)
                            if half == 0:
                                nc.vector.tensor_copy(
                                    acc_t[co][:, i0:i0 + ICH], ps_a[:])
                            else:
                                nc.vector.tensor_tensor(
                                    acc_t[co][:, i0:i0 + ICH],
                                    acc_t[co][:, i0:i0 + ICH], ps_a[:],
                                    alu.add)
                                # normalize into the fp8 DR tile right
                                # away so the projection is only one
                                # DVE op behind the last attnV chain
                                g, i = divmod(co, 2)
                                nc.vector.tensor_tensor(
                                    accn8_t[g][:, i, sl],
                                    acc_t[co][:, sl], rb[:, sl], alu.mult)
                        if half == 0:
                            continue
                        # finalize this chunk right away (overlaps the
                        # next chunk's attnV): accn8 = fp8(16*attnout),
                        # proj = DR(wp8, accn8) = 256*out,
                        # out = ps/256 + (x + bpd)
                        for co in range(CT):
                            ps = att_ps.tile([128, 512], f32, tag="att")
                            for g in range(NG):
                                mm8(ps[:],
                                    wp8_t[g][:, :, co * 128:(co + 1) * 128],
                                    accn8_t[g][:, :, sl],
                                    g == 0, g == NG - 1)
                            ot = o_p.tile([128, 512], f32, tag="o")
                            nc.vector.scalar_tensor_tensor(
                                ot[:], ps[:], 1.0 / (WVS * WVS),
                                xb_t[co][:, sl],
                                op0=alu.mult, op1=alu.add)
                            nc.sync.dma_start(
                                out_d[co * 128:(co + 1) * 128, sl], ot[:])

    nc.compile()
    return nc


def kernel(x, gn_gamma, gn_beta, wq, bq, wk, bk, wv, bv, wp, bp):
    import ml_dtypes
    from concourse import bass_utils

    if "nc" not in _CACHE:
        _CACHE["nc"] = _build()
    nc = _CACHE["nc"]

    x = np.asarray(x, np.float32)
    f = np.float32
    bf = ml_dtypes.bfloat16
    f8 = ml_dtypes.float8_e4m3
    wq32 = np.asarray(wq, f)
    wk32 = np.asarray(wk, f)
    m0T = (wq32.T @ wk32).astype(bf)  # (wk^T wq)^T
    qkbc = (wk32.T @ np.asarray(bq, f)).reshape(C, 1)
    wvT = np.asarray(wv, f).T.astype(bf)
    wts2 = np.ascontiguousarray(np.concatenate([m0T, wvT], axis=1))
    wpd = np.ascontiguousarray((np.asarray(wp, f).T / WVS).astype(bf))
    bp_eff = (np.asarray(bp, f)
              + np.asarray(wp, f) @ np.asarray(bv, f)).reshape(C, 1)
    consts4 = np.ascontiguousarray(np.concatenate([
        np.asarray(gn_gamma, f).reshape(C, 1),
        np.asarray(gn_beta, f).reshape(C, 1),
        bp_eff, qkbc], axis=1))
    sel = np.zeros((128, 8), f)
    for p in range(128):
        sel[p, p // GS] = 1.0
    common = {
        "wts2": wts2, "wpd": wpd, "consts4": consts4,
        "sel": sel, "selT": np.ascontiguousarray(sel.T),
        "ones8": np.ones((128, 2, 128), f8),
    }
    in_maps = []
    for core in range(NCORES):
        b, s = divmod(core, 4)
        xb = x[b].reshape(C, N)
        # roll so this core's query block occupies columns 0:QS; key order
        # is permuted identically for all key-side tensors, and softmax
        # sums are order-invariant, so the program is core-independent.
        xperm = np.ascontiguousarray(np.roll(xb, -s * QS, axis=1))
        im = {**common, "xq": xperm[:, 0:QS].astype(bf)}
        for g in range(NG):
            x8g = xperm[g * 256:(g + 1) * 256].reshape(2, 128, N)
            im[f"x8_{g}"] = np.ascontiguousarray(
                x8g.transpose(1, 0, 2)).astype(f8)
        in_maps.append(im)

    res = bass_utils.run_bass_kernel_spmd(nc, in_maps,
                                          core_ids=list(range(NCORES)))
    _CACHE["last_result"] = res

    out = np.empty((B, C, N), np.float32)
    for core in range(NCORES):
        b, s = divmod(core, 4)
        out[b][:, s * QS:(s + 1) * QS] = res.results[core]["out"]
    return out.reshape(B, C, H, W)


# revision 39
# speedup vs baseline: 1.0538x; 1.0113x over previous
"""AttnBlock v12: fp8 DoubleRow everywhere, fp8 GroupNorm stats.

Sharding: core = (batch b in {0,1}) x (query slice s in {0..3}, 1024
queries).  Each core redundantly computes full V^T for its batch
(avoids cross-core collectives), attention for its query slice only.
The host rolls x columns per core so the core's query block is always
columns 0:1024 -- identical program, per-core data.

Math: h = GN(x) = A_c * x + B_c per channel (A, B from runtime stats).
  q = (wq*A)@x + (wq@B + bq)     weight columns scaled on device
  k = (wk*A)@x   (k-bias dropped: per-query-constant under softmax)
  v = (wv*A)@x + const; v-bias folded into the projection bias:
      bp_dev = bp + wp@bv + wp@(wv@B).

Pipeline (225.9us baseline -> 141.6us):
  - every matmul stage (scores, attn@V, V-production, softmax
    denominator, q-projection, final projection) runs fp8e4m3 with
    perf_mode=DoubleRow: 256-row contraction groups as [128, 2, free]
    tiles; weights prescaled x16 against fp8 subnormals (unwound in
    the q activation scale and the final residual add); exp shifted
    by -3 to fit e4m3's 240 max (cancels in softmax).
  - GroupNorm stats run directly on the fp8 x8 tiles (bn_stats on DVE
    for 6 of 8 groups per tile, ACT Identity/Square accum_out for the
    rest, tiny combines on GpSimd); only a 1MB bf16 query-slice is
    loaded for the residual add.
  - half-0 v-production hoisted before the bias folds (warms the PE,
    overlaps the DVE bias chain); one 8-bank PSUM layout for the whole
    post-stats region; psum->fp8 drains split ACT/DVE; the per-chunk
    finalize is fused into the half-1 loop with the softmax reciprocal
    broadcast hoisted right after the denominator.
"""

import os
import sys

import numpy as np

for _p in ("/opt/trn_rl_repo", "/root/.axon_site/_ro/trn_rl_repo"):
    if os.path.isdir(_p) and _p not in sys.path:
        sys.path.insert(0, _p)

B, C, H, W = 2, 512, 64, 64
N = H * W
G = 32
GS = C // G
EPS = 1e-6
NCORES = 8
QS = N // 4               # 1024 queries per core
NHALF = 2                 # key halves
JQ = N // NHALF           # 2048 keys per half
JT = JQ // 128            # 16 key tiles per half
KT2 = JT // 2             # 8 DoubleRow key groups per half
ICH = 512                 # query chunk
NCH = QS // ICH           # 2 chunks
CT = C // 128             # 4 channel tiles
NG = 2                    # DoubleRow channel groups (256 ch each)
SCALE = float(C) ** -0.5
WVS = 16.0                # wv / m0 / wp prescale into fp8
ESHIFT = -3.0             # exp(s + ESHIFT): keep e in fp8e4m3 range
                          # (max scaled score ~7.2; e4m3 max normal 240);
                          # a constant shift cancels in the softmax ratio
NDVE = 6                  # of 8 per-tile stat groups on DVE (rest ACT)

_CACHE = {}


def _build():
    import contextlib

    import concourse.mybir as mybir
    import concourse.tile as tile
    from concourse import bacc
    from concourse.alu_op_type import AluOpType as alu

    f32 = mybir.dt.float32
    bf16 = mybir.dt.bfloat16
    f8 = mybir.dt.float8e4
    AF = mybir.ActivationFunctionType
    PM = mybir.MatmulPerfMode

    nc = bacc.Bacc("TRN2", target_bir_lowering=False, debug=False,
                   num_devices=NCORES)

    xqd = nc.dram_tensor("xq", [C, QS], bf16, kind="ExternalInput").ap()
    x8d = [nc.dram_tensor(f"x8_{g}", [128, 2, N], f8,
                          kind="ExternalInput").ap() for g in range(NG)]
    # wts2 = [m0T | wvT]; wpd = wp.T/16; consts4 = [gamma | beta |
    # bp_eff | qkbc] as columns
    wts2 = nc.dram_tensor("wts2", [C, 2 * C], bf16, kind="ExternalInput").ap()
    wpd = nc.dram_tensor("wpd", [C, C], bf16, kind="ExternalInput").ap()
    consts4 = nc.dram_tensor("consts4", [C, 4], f32,
                             kind="ExternalInput").ap()
    sel = nc.dram_tensor("sel", [128, 8], f32, kind="ExternalInput").ap()
    selT = nc.dram_tensor("selT", [8, 128], f32, kind="ExternalInput").ap()
    ones8 = nc.dram_tensor("ones8", [128, 2, 128], f8,
                           kind="ExternalInput").ap()
    out_d = nc.dram_tensor("out", [C, QS], bf16, kind="ExternalOutput").ap()

    def mm(ps, lhsT, rhs, start, stop):
        nc.tensor.matmul(ps, lhsT, rhs, start=start, stop=stop)

    def mm8(ps, lhsT, rhs, start, stop):
        nc.tensor.matmul(ps, lhsT, rhs, start=start, stop=stop,
                         perf_mode=PM.DoubleRow)

    with tile.TileContext(nc) as tc:
        outer = contextlib.ExitStack()
        with outer:
            cpool = outer.enter_context(tc.tile_pool(name="const", bufs=1))
            x_p = outer.enter_context(tc.tile_pool(name="xbf", bufs=1))
            x8_p = outer.enter_context(tc.tile_pool(name="x8", bufs=1))
            acc_p = outer.enter_context(tc.tile_pool(name="acc", bufs=1))
            w_p = outer.enter_context(tc.tile_pool(name="wts", bufs=1))
            q8_p = outer.enter_context(tc.tile_pool(name="q8", bufs=1))
            v8_p = outer.enter_context(tc.tile_pool(name="v8", bufs=2 * KT2))
            e8_p = outer.enter_context(tc.tile_pool(name="e8",
                                                    bufs=2 * KT2 + 2))
            f_p = outer.enter_context(tc.tile_pool(name="fin", bufs=1))

            # ---- Sync ring, in transfer-priority order: x8 in column
            # ---- chunks (stats run on the fp8 x directly), then the
            # ---- bf16 query-slice (residual), then m0/wv, then wp ----
            x8_t = []
            for g in range(NG):
                xt8 = x8_p.tile([128, 2, N], f8, tag=f"x8_{g}",
                                name=f"x8_{g}")
                for c in range(N // 1024):
                    nc.sync.dma_start(
                        xt8[:, :, c * 1024:(c + 1) * 1024],
                        x8d[g][:, :, c * 1024:(c + 1) * 1024])
                x8_t.append(xt8)

            def xsl(ci, start, size):
                # stats input: channel tile ci lives in x8 group ci//2,
                # pair-slot ci%2 (channel = 256*(ci//2) + 128*(ci%2) + p)
                return x8_t[ci // 2][:, ci % 2, start:start + size]

            xq_t = []
            for t in range(CT):
                xt = x_p.tile([128, QS], bf16, tag=f"xq{t}",
                              name=f"xq{t}")
                nc.sync.dma_start(xt[:], xqd[t * 128:(t + 1) * 128, :])
                xq_t.append(xt)
            wts_t = []
            for t in range(CT):
                wt = w_p.tile([128, 2 * C], bf16, tag=f"wts{t}")
                nc.sync.dma_start(wt[:], wts2[t * 128:(t + 1) * 128, :])
                wts_t.append(wt)
            m0_t = [wts_t[t][:, 0:C] for t in range(CT)]
            wv_t = [wts_t[t][:, C:2 * C] for t in range(CT)]
            wp_t = []
            for t in range(CT):
                wt = w_p.tile([128, C], bf16, tag=f"wp{t}")
                nc.sync.dma_start(wt[:], wpd[t * 128:(t + 1) * 128, :])
                wp_t.append(wt)

            # ---- tiny consts on the GpSimd ring (no bandwidth impact) ----
            c4_t = []
            for t in range(CT):
                c4 = cpool.tile([128, 4], f32, tag=f"c4_{t}")
                nc.gpsimd.dma_start(c4[:], consts4[t * 128:(t + 1) * 128, :])
                c4_t.append(c4)
            gam_t = [c4_t[t][:, 0:1] for t in range(CT)]
            bet_t = [c4_t[t][:, 1:2] for t in range(CT)]
            bp_t = [c4_t[t][:, 2:3] for t in range(CT)]
            qkbc_t = [c4_t[t][:, 3:4] for t in range(CT)]
            sel_t = cpool.tile([128, 8], f32, tag="sel")
            nc.gpsimd.dma_start(sel_t[:], sel[:])
            selT_t = cpool.tile([8, 128], f32, tag="selT")
            nc.gpsimd.dma_start(selT_t[:], selT[:])
            ones8_t = cpool.tile([128, 2, 128], f8, tag="ones8")
            nc.gpsimd.dma_start(ones8_t[:], ones8[:])
            esh_t = cpool.tile([128, 1], f32, tag="esh")
            nc.vector.memset(esh_t[:], ESHIFT)
            cfrac_t = cpool.tile([128, 1], f32, tag="cfrac")
            nc.vector.memset(cfrac_t[:], NDVE / 8.0)
            cinvN_t = cpool.tile([128, 1], f32, tag="cinvN")
            nc.vector.memset(cinvN_t[:], 1.0 / N)

            den_acc = acc_p.tile([1, QS], f32, tag="den")
            recip = acc_p.tile([1, QS], f32, tag="recip")
            acc_t = [acc_p.tile([128, QS], f32, tag=f"acc{t}",
                                name=f"acc{t}") for t in range(CT)]

            # ---- GroupNorm stats: DVE bn_stats (groups 0..NDVE-1) in
            # ---- parallel with ACT Identity/Square accum (the rest)
            with tc.tile_pool(name="small", bufs=1) as sm_p, \
                 tc.tile_pool(name="scr", bufs=2) as scr_p, \
                 tc.tile_pool(name="stat_ps", bufs=1, space="PSUM") as stat_ps, \
                 tc.tile_pool(name="ab_ps", bufs=2, space="PSUM") as ab_ps:
                ps_st = stat_ps.tile([8, 8], f32, tag="st")
                for t in range(CT):
                    st = sm_p.tile([128, NDVE, 6], f32, tag=f"bnst{t}")
                    for g in range(NDVE):
                        nc.vector.bn_stats(st[:, g, :],
                                           xsl(t, g * 512, 512))
                    ag = sm_p.tile([128, 2], f32, tag=f"bnag{t}")
                    nc.vector.bn_aggr(ag[:], st[:])
                    nact = 8 - NDVE
                    sx = sm_p.tile([128, nact], f32, tag=f"sx{t}")
                    sq = sm_p.tile([128, nact], f32, tag=f"sq{t}")
                    for k in range(nact):
                        g = NDVE + k
                        scr = scr_p.tile([128, 512], bf16, tag="scr")
                        nc.scalar.activation(scr[:], xsl(t, g * 512, 512),
                                             AF.Identity,
                                             accum_out=sx[:, k:k + 1])
                        scr2 = scr_p.tile([128, 512], bf16, tag="scr")
                        nc.scalar.activation(scr2[:], xsl(t, g * 512, 512),
                                             AF.Square,
                                             accum_out=sq[:, k:k + 1])
                    # combine into mean over 4096 and E[x^2] over 4096 --
                    # tensor_tensor-only ops on the otherwise-idle GpSimd
                    # (Pool rejects TensorScalar) so DVE stays on
                    # bn_stats for the next tile
                    u = sm_p.tile([128, 1], f32, tag=f"u{t}")
                    if nact > 1:
                        nc.gpsimd.tensor_tensor(u[:], sx[:, 0:1],
                                                sx[:, 1:2], alu.add)
                        for k in range(2, nact):
                            nc.gpsimd.tensor_tensor(u[:], u[:],
                                                    sx[:, k:k + 1], alu.add)
                    else:
                        nc.gpsimd.tensor_copy(u[:], sx[:])
                    mean_t = sm_p.tile([128, 1], f32, tag=f"mean{t}")
                    nc.gpsimd.tensor_tensor(mean_t[:], ag[:, 0:1],
                                            cfrac_t[:], alu.mult)
                    nc.gpsimd.tensor_tensor(u[:], u[:], cinvN_t[:],
                                            alu.mult)
                    nc.gpsimd.tensor_tensor(mean_t[:], mean_t[:], u[:],
                                            alu.add)
                    v = sm_p.tile([128, 1], f32, tag=f"v{t}")
                    if nact > 1:
                        nc.gpsimd.tensor_tensor(v[:], sq[:, 0:1],
                                                sq[:, 1:2], alu.add)
                        for k in range(2, nact):
                            nc.gpsimd.tensor_tensor(v[:], v[:],
                                                    sq[:, k:k + 1], alu.add)
                    else:
                        nc.gpsimd.tensor_copy(v[:], sq[:])
                    s2_t = sm_p.tile([128, 1], f32, tag=f"s2{t}")
                    nc.gpsimd.tensor_tensor(s2_t[:], ag[:, 0:1], ag[:, 0:1],
                                            alu.mult)
                    nc.gpsimd.tensor_tensor(s2_t[:], s2_t[:], ag[:, 1:2],
                                            alu.add)
                    nc.gpsimd.tensor_tensor(s2_t[:], s2_t[:], cfrac_t[:],
                                            alu.mult)
                    nc.gpsimd.tensor_tensor(v[:], v[:], cinvN_t[:],
                                            alu.mult)
                    nc.gpsimd.tensor_tensor(s2_t[:], s2_t[:], v[:],
                                            alu.add)
                    nc.tensor.matmul(ps_st[:, t:t + 1], sel_t[:], mean_t[:],
                                     start=True, stop=True)
                    nc.tensor.matmul(ps_st[:, 4 + t:5 + t], sel_t[:],
                                     s2_t[:], start=True, stop=True)
                st_sb = sm_p.tile([8, 8], f32, tag="st_sb")
                nc.vector.tensor_copy(st_sb[:], ps_st[:])
                mean = sm_p.tile([8, 4], f32, tag="mean")
                nc.vector.tensor_scalar(mean[:], st_sb[:, 0:4],
                                        1.0 / GS, None, op0=alu.mult)
                msq = sm_p.tile([8, 4], f32, tag="msq")
                nc.vector.tensor_scalar(msq[:], st_sb[:, 4:8],
                                        1.0 / GS, None, op0=alu.mult)
                var = sm_p.tile([8, 4], f32, tag="var")
                nc.vector.tensor_tensor(var[:], mean[:], mean[:], alu.mult)
                nc.vector.tensor_tensor(var[:], msq[:], var[:], alu.subtract)
                nc.vector.tensor_scalar(var[:], var[:], EPS, None, op0=alu.add)
                sd = sm_p.tile([8, 4], f32, tag="sd")
                nc.scalar.activation(sd[:], var[:], AF.Sqrt)
                rstd = sm_p.tile([8, 4], f32, tag="rstd")
                nc.vector.reciprocal(rstd[:], sd[:])
                A_t, A16_t, Ai16_t, Bb_t = [], [], [], []
                for t in range(CT):
                    ps_ab = ab_ps.tile([128, 2], f32, tag="ab")
                    nc.tensor.matmul(ps_ab[:, 0:1], selT_t[:],
                                     rstd[:, t:t + 1], start=True, stop=True)
                    nc.tensor.matmul(ps_ab[:, 1:2], selT_t[:],
                                     mean[:, t:t + 1], start=True, stop=True)
                    ab = cpool.tile([128, 2], f32, tag=f"ab{t}")
                    nc.vector.tensor_copy(ab[:], ps_ab[:])
                    At = cpool.tile([128, 1], f32, tag=f"A{t}")
                    nc.vector.tensor_tensor(At[:], ab[:, 0:1], gam_t[t],
                                            alu.mult)
                    At16 = cpool.tile([128, 1], f32, tag=f"A16_{t}")
                    nc.vector.tensor_scalar(At16[:], At[:], WVS, None,
                                            op0=alu.mult)
                    Ai16 = cpool.tile([128, 1], f32, tag=f"Ai16_{t}")
                    nc.vector.tensor_scalar(Ai16[:], At[:], 1.0 / WVS, None,
                                            op0=alu.mult)
                    Bt = cpool.tile([128, 1], f32, tag=f"B{t}")
                    nc.vector.tensor_tensor(Bt[:], ab[:, 1:2], At[:], alu.mult)
                    nc.vector.tensor_tensor(Bt[:], bet_t[t], Bt[:],
                                            alu.subtract)
                    Bb = cpool.tile([128, 1], bf16, tag=f"Bb{t}")
                    nc.vector.tensor_copy(Bb[:], Bt[:])
                    A_t.append(At)
                    A16_t.append(At16)
                    Ai16_t.append(Ai16)
                    Bb_t.append(Bb)

                # fp8 DR weight tiles (prescaled x16; t = 2g + i), on DVE:
                #   wv8 = fp8(A16*wv rows), m08 = fp8(A16*m0 rows)
                # (wp8 is converted later -- it is only needed at the
                # finalize, and converting it here queues DVE work ahead
                # of the q8 writes that gate the first scores)
                wv8_t, m08_t = [], []
                for g in range(NG):
                    w8 = w_p.tile([128, 2, C], f8, tag=f"wv8_{g}")
                    m8 = w_p.tile([128, 2, C], f8, tag=f"m08_{g}")
                    for i in range(2):
                        t = 2 * g + i
                        nc.vector.tensor_scalar(w8[:, i, :], wv_t[t],
                                                A16_t[t][:], None,
                                                op0=alu.mult)
                        nc.vector.tensor_scalar(m8[:, i, :], m0_t[t],
                                                A16_t[t][:], None,
                                                op0=alu.mult)
                    wv8_t.append(w8)
                    m08_t.append(m8)

            # ---- post-stats region: one PSUM layout (3 + 3 + small) ----
            with tc.tile_pool(name="mm_ps", bufs=3, space="PSUM") as mm_ps, \
                 tc.tile_pool(name="att_ps", bufs=3, space="PSUM") as att_ps, \
                 tc.tile_pool(name="sm2_ps", bufs=1, space="PSUM") as smp:

                def vprod(half):
                    j0 = half * JQ
                    v8_t = []
                    for jt in range(JT):
                        ps = mm_ps.tile([128, 512], f32, tag="mm")
                        for g in range(NG):
                            mm8(ps[:],
                                x8_t[g][:, :, j0 + jt * 128:
                                        j0 + (jt + 1) * 128],
                                wv8_t[g][:], g == 0, g == NG - 1)
                        kt2, slot = divmod(jt, 2)
                        if slot == 0:
                            vt = v8_p.tile([128, 2, C], f8, tag="v8")
                            v8_t.append(vt)
                        if jt % 4 < 2:
                            nc.scalar.copy(v8_t[kt2][:, slot, :], ps[:])
                        else:
                            nc.vector.tensor_copy(v8_t[kt2][:, slot, :],
                                                  ps[:])
                    return v8_t

                # half-0 V first: only needs wv8 + x8; warms the PE while
                # the bias/q chain resolves
                v8_half0 = vprod(0)

                # bias terms from RAW weights:
                #   qkb = M0@B + wk^T bq (host const);  Abias = A*qkb
                #   tv  = wv@B  (for the projection-bias fold)
                abias_t, tvb_t = [], []
                for co in range(CT):
                    ps_b = smp.tile([128, 2], f32, tag="bb")
                    for ci in range(CT):
                        mm(ps_b[:, 0:1],
                           m0_t[ci][:, co * 128:(co + 1) * 128], Bb_t[ci][:],
                           ci == 0, ci == CT - 1)
                    for ci in range(CT):
                        mm(ps_b[:, 1:2],
                           wv_t[ci][:, co * 128:(co + 1) * 128], Bb_t[ci][:],
                           ci == 0, ci == CT - 1)
                    ab2 = cpool.tile([128, 1], f32, tag=f"abias{co}")
                    nc.vector.tensor_tensor(ab2[:], ps_b[:, 0:1],
                                            qkbc_t[co], alu.add)
                    nc.vector.tensor_tensor(ab2[:], ab2[:], A_t[co][:],
                                            alu.mult)
                    abias_t.append(ab2)
                    tvb = cpool.tile([128, 1], bf16, tag=f"tvb{co}")
                    nc.vector.tensor_copy(tvb[:], ps_b[:, 1:2])
                    tvb_t.append(tvb)

                # qk projection (fp8 DR) -> fp8 DR tiles q8[g][:, i, :]
                # psum carries 16*q (m08 prescale); scale back with A/16
                q8_t = [q8_p.tile([128, 2, QS], f8, tag=f"q8_{g}",
                                  name=f"q8_{g}") for g in range(NG)]
                for co in range(CT):
                    g, i = divmod(co, 2)
                    for nn in range(QS // 512):
                        ps = mm_ps.tile([128, 512], f32, tag="mm")
                        for gi in range(NG):
                            mm8(ps[:],
                                m08_t[gi][:, :, co * 128:(co + 1) * 128],
                                x8_t[gi][:, :, nn * 512:(nn + 1) * 512],
                                gi == 0, gi == NG - 1)
                        nc.vector.tensor_scalar(
                            q8_t[g][:, i, nn * 512:(nn + 1) * 512],
                            ps[:], Ai16_t[co][:], abias_t[co][:],
                            op0=alu.mult, op1=alu.add)

                # device projection bias bpd = 16*(wp/16)@tv + bp_eff, and
                # xb = x_residual + bpd so the finalize needs one STT
                bpd_t, xb_t = [], []
                for co in range(CT):
                    ps_u = smp.tile([128, 1], f32, tag="u")
                    for ci in range(CT):
                        mm(ps_u[:], wp_t[ci][:, co * 128:(co + 1) * 128],
                           tvb_t[ci][:], ci == 0, ci == CT - 1)
                    bpd = f_p.tile([128, 1], f32, tag=f"bpd{co}")
                    nc.vector.scalar_tensor_tensor(
                        bpd[:], ps_u[:], WVS, bp_t[co],
                        op0=alu.mult, op1=alu.add)
                    bpd_t.append(bpd)
                    xb = f_p.tile([128, QS], bf16, tag=f"xb{co}")
                    nc.vector.tensor_scalar(xb[:], xq_t[co][:],
                                            bpd[:], None, op0=alu.add)
                    xb_t.append(xb)

                # wp8 = fp8(16 * wp.T) [host sent wp.T/16 -> scale 256];
                # emitted after the q-side DVE work (finalize-only use)
                wp8_t = []
                for g in range(NG):
                    p8 = w_p.tile([128, 2, C], f8, tag=f"wp8_{g}")
                    for i in range(2):
                        t = 2 * g + i
                        nc.vector.tensor_scalar(p8[:, i, :], wp_t[t][:],
                                                WVS * WVS, None,
                                                op0=alu.mult)
                    wp8_t.append(p8)

                # ---- attention over key halves (fp8 DoubleRow) ----
                o_p = outer.enter_context(tc.tile_pool(name="outp", bufs=3))
                rb = f_p.tile([128, QS], f32, tag="rb")
                accn8_t = [f_p.tile([128, 2, QS], f8, tag=f"accn8_{g}",
                                    name=f"accn8_{g}") for g in range(NG)]
                for half in range(NHALF):
                    j0 = half * JQ
                    v8_t = v8_half0 if half == 0 else vprod(1)

                    # scores + exp for BOTH chunks first, so the last
                    # chunk's exps (ACT) overlap the first chunk's attnV
                    e8_c = []
                    for ch in range(NCH):
                        i0 = ch * ICH
                        e8_t = []
                        for jt in range(JT):
                            ps = mm_ps.tile([128, ICH], f32, tag="mm")
                            for g in range(NG):
                                mm8(ps[:],
                                    x8_t[g][:, :, j0 + jt * 128:
                                            j0 + (jt + 1) * 128],
                                    q8_t[g][:, :, i0:i0 + ICH],
                                    g == 0, g == NG - 1)
                            kt2, slot = divmod(jt, 2)
                            if slot == 0:
                                et = e8_p.tile([128, 2, ICH], f8, tag="e8")
                                e8_t.append(et)
                            nc.scalar.activation(e8_t[kt2][:, slot, :],
                                                 ps[:], AF.Exp, scale=SCALE,
                                                 bias=esh_t[:])
                        e8_c.append(e8_t)

                    for ch in range(NCH):
                        i0 = ch * ICH
                        e8_t = e8_c[ch]
                        # denominator: all-ones stationary (every output
                        # partition carries the same key-sum; row 0 used)
                        ps_d = att_ps.tile([128, ICH], f32, tag="att")
                        for kt2 in range(KT2):
                            mm8(ps_d[:], ones8_t[:], e8_t[kt2][:],
                                kt2 == 0, kt2 == KT2 - 1)
                        if half == 0:
                            nc.vector.tensor_copy(den_acc[:, i0:i0 + ICH],
                                                  ps_d[0:1, :])
                        else:
                            nc.vector.tensor_tensor(den_acc[:, i0:i0 + ICH],
                                                    den_acc[:, i0:i0 + ICH],
                                                    ps_d[0:1, :], alu.add)
                            nc.vector.reciprocal(recip[:, i0:i0 + ICH],
                                                 den_acc[:, i0:i0 + ICH])
                            # broadcast early: depends only on the denom
                            nc.gpsimd.partition_broadcast(
                                rb[:, i0:i0 + ICH], recip[:, i0:i0 + ICH])
                        for co in range(CT):
                            ps_a = att_ps.tile([128, ICH], f32, tag="att")
                            for kt2 in range(KT2):
                                mm8(ps_a[:],
                                    v8_t[kt2][:, :, co * 128:(co + 1) * 128],
                                    e8_t[kt2][:], kt2 == 0, kt2 == KT2 - 1)
                            sl = slice(i0, i0 + ICH)
                            if half == 0:
                                nc.vector.tensor_copy(
                                    acc_t[co][:, i0:i0 + ICH], ps_a[:])
                            else:
                                nc.vector.tensor_tensor(
                                    acc_t[co][:, i0:i0 + ICH],
                                    acc_t[co][:, i0:i0 + ICH], ps_a[:],
                                    alu.add)
                                # normalize into the fp8 DR tile right
                                # away so the projection is only one
                                # DVE op behind the last attnV chain
                                g, i = divmod(co, 2)
                                nc.vector.tensor_tensor(
                                    accn8_t[g][:, i, sl],
                                    acc_t[co][:, sl], rb[:, sl], alu.mult)
                        if half == 0:
                            continue
                        # finalize this chunk right away (overlaps the
                        # next chunk's attnV): accn8 = fp8(16*attnout),
                        # proj = DR(wp8, accn8) = 256*out,
                        # out = ps/256 + (x + bpd)
                        for co in range(CT):
                            ps = att_ps.tile([128, 512], f32, tag="att")
                            for g in range(NG):
                                mm8(ps[:],
                                    wp8_t[g][:, :, co * 128:(co + 1) * 128],
                                    accn8_t[g][:, :, sl],
                                    g == 0, g == NG - 1)
                            ot = o_p.tile([128, 512], bf16, tag="o")
                            nc.vector.scalar_tensor_tensor(
                                ot[:], ps[:], 1.0 / (WVS * WVS),
                                xb_t[co][:, sl],
                                op0=alu.mult, op1=alu.add)
                            nc.sync.dma_start(
                                out_d[co * 128:(co + 1) * 128, sl], ot[:])

    nc.compile()
    return nc


def kernel(x, gn_gamma, gn_beta, wq, bq, wk, bk, wv, bv, wp, bp):
    import ml_dtypes
    from concourse import bass_utils

    if "nc" not in _CACHE:
        _CACHE["nc"] = _build()
    nc = _CACHE["nc"]

    x = np.asarray(x, np.float32)
    f = np.float32
    bf = ml_dtypes.bfloat16
    f8 = ml_dtypes.float8_e4m3
    wq32 = np.asarray(wq, f)
    wk32 = np.asarray(wk, f)
    m0T = (wq32.T @ wk32).astype(bf)  # (wk^T wq)^T
    qkbc = (wk32.T @ np.asarray(bq, f)).reshape(C, 1)
    wvT = np.asarray(wv, f).T.astype(bf)
    wts2 = np.ascontiguousarray(np.concatenate([m0T, wvT], axis=1))
    wpd = np.ascontiguousarray((np.asarray(wp, f).T / WVS).astype(bf))
    bp_eff = (np.asarray(bp, f)
              + np.asarray(wp, f) @ np.asarray(bv, f)).reshape(C, 1)
    consts4 = np.ascontiguousarray(np.concatenate([
        np.asarray(gn_gamma, f).reshape(C, 1),
        np.asarray(gn_beta, f).reshape(C, 1),
        bp_eff, qkbc], axis=1))
    sel = np.zeros((128, 8), f)
    for p in range(128):
        sel[p, p // GS] = 1.0
    common = {
        "wts2": wts2, "wpd": wpd, "consts4": consts4,
        "sel": sel, "selT": np.ascontiguousarray(sel.T),
        "ones8": np.ones((128, 2, 128), f8),
    }
    in_maps = []
    for core in range(NCORES):
        b, s = divmod(core, 4)
        xb = x[b].reshape(C, N)
        # roll so this core's query block occupies columns 0:QS; key order
        # is permuted identically for all key-side tensors, and softmax
        # sums are order-invariant, so the program is core-independent.
        xperm = np.ascontiguousarray(np.roll(xb, -s * QS, axis=1))
        im = {**common, "xq": xperm[:, 0:QS].astype(bf)}
        for g in range(NG):
            x8g = xperm[g * 256:(g + 1) * 256].reshape(2, 128, N)
            im[f"x8_{g}"] = np.ascontiguousarray(
                x8g.transpose(1, 0, 2)).astype(f8)
        in_maps.append(im)

    res = bass_utils.run_bass_kernel_spmd(nc, in_maps,
                                          core_ids=list(range(NCORES)))
    _CACHE["last_result"] = res

    out = np.empty((B, C, N), np.float32)
    for core in range(NCORES):
        b, s = divmod(core, 4)
        out[b][:, s * QS:(s + 1) * QS] = np.asarray(
            res.results[core]["out"], np.float32)
    return out.reshape(B, C, H, W)


# revision 40
# speedup vs baseline: 1.0538x; 1.0000x over previous
"""AttnBlock v12: fp8 DoubleRow everywhere, fp8 GroupNorm stats.

Sharding: core = (batch b in {0,1}) x (query slice s in {0..3}, 1024
queries).  Each core redundantly computes full V^T for its batch
(avoids cross-core collectives), attention for its query slice only.
The host rolls x columns per core so the core's query block is always
columns 0:1024 -- identical program, per-core data.

Math: h = GN(x) = A_c * x + B_c per channel (A, B from runtime stats).
  q = (wq*A)@x + (wq@B + bq)     weight columns scaled on device
  k = (wk*A)@x   (k-bias dropped: per-query-constant under softmax)
  v = (wv*A)@x + const; v-bias folded into the projection bias:
      bp_dev = bp + wp@bv + wp@(wv@B).

Pipeline (225.9us baseline -> 141.6us):
  - every matmul stage (scores, attn@V, V-production, softmax
    denominator, q-projection, final projection) runs fp8e4m3 with
    perf_mode=DoubleRow: 256-row contraction groups as [128, 2, free]
    tiles; weights prescaled x16 against fp8 subnormals (unwound in
    the q activation scale and the final residual add); exp shifted
    by -3 to fit e4m3's 240 max (cancels in softmax).
  - GroupNorm stats run directly on the fp8 x8 tiles (bn_stats on DVE
    for 6 of 8 groups per tile, ACT Identity/Square accum_out for the
    rest, tiny combines on GpSimd); only a 1MB bf16 query-slice is
    loaded for the residual add.
  - half-0 v-production hoisted before the bias folds (warms the PE,
    overlaps the DVE bias chain); one 8-bank PSUM layout for the whole
    post-stats region; psum->fp8 drains split ACT/DVE; the per-chunk
    finalize is fused into the half-1 loop with the softmax reciprocal
    broadcast hoisted right after the denominator.
"""

import os
import sys

import numpy as np

for _p in ("/opt/trn_rl_repo", "/root/.axon_site/_ro/trn_rl_repo"):
    if os.path.isdir(_p) and _p not in sys.path:
        sys.path.insert(0, _p)

B, C, H, W = 2, 512, 64, 64
N = H * W
G = 32
GS = C // G
EPS = 1e-6
NCORES = 8
QS = N // 4               # 1024 queries per core
NHALF = 2                 # key halves
JQ = N // NHALF           # 2048 keys per half
JT = JQ // 128            # 16 key tiles per half
KT2 = JT // 2             # 8 DoubleRow key groups per half
ICH = 512                 # query chunk
NCH = QS // ICH           # 2 chunks
CT = C // 128             # 4 channel tiles
NG = 2                    # DoubleRow channel groups (256 ch each)
SCALE = float(C) ** -0.5
WVS = 16.0                # wv / m0 / wp prescale into fp8
ESHIFT = -3.0             # exp(s + ESHIFT): keep e in fp8e4m3 range
                          # (max scaled score ~7.2; e4m3 max normal 240);
                          # a constant shift cancels in the softmax ratio
NDVE = 6                  # of 8 per-tile stat groups on DVE (rest ACT)

_CACHE = {}


def _build():
    import contextlib

    import concourse.mybir as mybir
    import concourse.tile as tile
    from concourse import bacc
    from concourse.alu_op_type import AluOpType as alu

    f32 = mybir.dt.float32
    bf16 = mybir.dt.bfloat16
    f8 = mybir.dt.float8e4
    AF = mybir.ActivationFunctionType
    PM = mybir.MatmulPerfMode

    nc = bacc.Bacc("TRN2", target_bir_lowering=False, debug=False,
                   num_devices=NCORES)

    xqd = nc.dram_tensor("xq", [C, QS], bf16, kind="ExternalInput").ap()
    x8d = [nc.dram_tensor(f"x8_{g}", [128, 2, N], f8,
                          kind="ExternalInput").ap() for g in range(NG)]
    # wts2 = [m0T | wvT]; wpd = wp.T/16; consts4 = [gamma | beta |
    # bp_eff | qkbc] as columns
    wts2 = nc.dram_tensor("wts2", [C, 2 * C], bf16, kind="ExternalInput").ap()
    wpd = nc.dram_tensor("wpd", [C, C], bf16, kind="ExternalInput").ap()
    consts4 = nc.dram_tensor("consts4", [C, 4], f32,
                             kind="ExternalInput").ap()
    sel = nc.dram_tensor("sel", [128, 8], f32, kind="ExternalInput").ap()
    selT = nc.dram_tensor("selT", [8, 128], f32, kind="ExternalInput").ap()
    ones8 = nc.dram_tensor("ones8", [128, 2, 128], f8,
                           kind="ExternalInput").ap()
    out_d = nc.dram_tensor("out", [C, QS], bf16, kind="ExternalOutput").ap()

    def mm(ps, lhsT, rhs, start, stop):
        nc.tensor.matmul(ps, lhsT, rhs, start=start, stop=stop)

    def mm8(ps, lhsT, rhs, start, stop):
        nc.tensor.matmul(ps, lhsT, rhs, start=start, stop=stop,
                         perf_mode=PM.DoubleRow)

    with tile.TileContext(nc) as tc:
        outer = contextlib.ExitStack()
        with outer:
            cpool = outer.enter_context(tc.tile_pool(name="const", bufs=1))
            x_p = outer.enter_context(tc.tile_pool(name="xbf", bufs=1))
            x8_p = outer.enter_context(tc.tile_pool(name="x8", bufs=1))
            acc_p = outer.enter_context(tc.tile_pool(name="acc", bufs=1))
            w_p = outer.enter_context(tc.tile_pool(name="wts", bufs=1))
            q8_p = outer.enter_context(tc.tile_pool(name="q8", bufs=1))
            v8_p = outer.enter_context(tc.tile_pool(name="v8", bufs=2 * KT2))
            e8_p = outer.enter_context(tc.tile_pool(name="e8",
                                                    bufs=2 * KT2 + 2))
            f_p = outer.enter_context(tc.tile_pool(name="fin", bufs=1))

            # ---- Sync ring, in transfer-priority order: x8 in column
            # ---- chunks (stats run on the fp8 x directly), then the
            # ---- bf16 query-slice (residual), then m0/wv, then wp ----
            x8_t = []
            for g in range(NG):
                xt8 = x8_p.tile([128, 2, N], f8, tag=f"x8_{g}",
                                name=f"x8_{g}")
                for c in range(N // 1024):
                    nc.sync.dma_start(
                        xt8[:, :, c * 1024:(c + 1) * 1024],
                        x8d[g][:, :, c * 1024:(c + 1) * 1024])
                x8_t.append(xt8)

            def xsl(ci, start, size):
                # stats input: channel tile ci lives in x8 group ci//2,
                # pair-slot ci%2 (channel = 256*(ci//2) + 128*(ci%2) + p)
                return x8_t[ci // 2][:, ci % 2, start:start + size]

            xq_t = []
            for t in range(CT):
                xt = x_p.tile([128, QS], bf16, tag=f"xq{t}",
                              name=f"xq{t}")
                nc.sync.dma_start(xt[:], xqd[t * 128:(t + 1) * 128, :])
                xq_t.append(xt)
            wts_t = []
            for t in range(CT):
                wt = w_p.tile([128, 2 * C], bf16, tag=f"wts{t}")
                nc.sync.dma_start(wt[:], wts2[t * 128:(t + 1) * 128, :])
                wts_t.append(wt)
            m0_t = [wts_t[t][:, 0:C] for t in range(CT)]
            wv_t = [wts_t[t][:, C:2 * C] for t in range(CT)]
            wp_t = []
            for t in range(CT):
                wt = w_p.tile([128, C], bf16, tag=f"wp{t}")
                nc.sync.dma_start(wt[:], wpd[t * 128:(t + 1) * 128, :])
                wp_t.append(wt)

            # ---- tiny consts on the GpSimd ring (no bandwidth impact) ----
            c4_t = []
            for t in range(CT):
                c4 = cpool.tile([128, 4], f32, tag=f"c4_{t}")
                nc.gpsimd.dma_start(c4[:], consts4[t * 128:(t + 1) * 128, :])
                c4_t.append(c4)
            gam_t = [c4_t[t][:, 0:1] for t in range(CT)]
            bet_t = [c4_t[t][:, 1:2] for t in range(CT)]
            bp_t = [c4_t[t][:, 2:3] for t in range(CT)]
            qkbc_t = [c4_t[t][:, 3:4] for t in range(CT)]
            sel_t = cpool.tile([128, 8], f32, tag="sel")
            nc.gpsimd.dma_start(sel_t[:], sel[:])
            selT_t = cpool.tile([8, 128], f32, tag="selT")
            nc.gpsimd.dma_start(selT_t[:], selT[:])
            ones8_t = cpool.tile([128, 2, 128], f8, tag="ones8")
            nc.gpsimd.dma_start(ones8_t[:], ones8[:])
            esh_t = cpool.tile([128, 1], f32, tag="esh")
            nc.vector.memset(esh_t[:], ESHIFT)
            cfrac_t = cpool.tile([128, 1], f32, tag="cfrac")
            nc.vector.memset(cfrac_t[:], NDVE / 8.0)
            cinvN_t = cpool.tile([128, 1], f32, tag="cinvN")
            nc.vector.memset(cinvN_t[:], 1.0 / N)

            den_acc = acc_p.tile([1, QS], f32, tag="den")
            recip = acc_p.tile([1, QS], f32, tag="recip")
            acc_t = [acc_p.tile([128, QS], f32, tag=f"acc{t}",
                                name=f"acc{t}") for t in range(CT)]

            # ---- GroupNorm stats: DVE bn_stats (groups 0..NDVE-1) in
            # ---- parallel with ACT Identity/Square accum (the rest)
            with tc.tile_pool(name="small", bufs=1) as sm_p, \
                 tc.tile_pool(name="scr", bufs=2) as scr_p, \
                 tc.tile_pool(name="stat_ps", bufs=1, space="PSUM") as stat_ps, \
                 tc.tile_pool(name="ab_ps", bufs=2, space="PSUM") as ab_ps:
                ps_st = stat_ps.tile([8, 8], f32, tag="st")
                for t in range(CT):
                    st = sm_p.tile([128, NDVE, 6], f32, tag=f"bnst{t}")
                    for g in range(NDVE):
                        nc.vector.bn_stats(st[:, g, :],
                                           xsl(t, g * 512, 512))
                    ag = sm_p.tile([128, 2], f32, tag=f"bnag{t}")
                    nc.vector.bn_aggr(ag[:], st[:])
                    nact = 8 - NDVE
                    sx = sm_p.tile([128, nact], f32, tag=f"sx{t}")
                    sq = sm_p.tile([128, nact], f32, tag=f"sq{t}")
                    for k in range(nact):
                        g = NDVE + k
                        scr = scr_p.tile([128, 512], bf16, tag="scr")
                        nc.scalar.activation(scr[:], xsl(t, g * 512, 512),
                                             AF.Identity,
                                             accum_out=sx[:, k:k + 1])
                        scr2 = scr_p.tile([128, 512], bf16, tag="scr")
                        nc.scalar.activation(scr2[:], xsl(t, g * 512, 512),
                                             AF.Square,
                                             accum_out=sq[:, k:k + 1])
                    # combine into mean over 4096 and E[x^2] over 4096 --
                    # tensor_tensor-only ops on the otherwise-idle GpSimd
                    # (Pool rejects TensorScalar) so DVE stays on
                    # bn_stats for the next tile
                    u = sm_p.tile([128, 1], f32, tag=f"u{t}")
                    if nact > 1:
                        nc.gpsimd.tensor_tensor(u[:], sx[:, 0:1],
                                                sx[:, 1:2], alu.add)
                        for k in range(2, nact):
                            nc.gpsimd.tensor_tensor(u[:], u[:],
                                                    sx[:, k:k + 1], alu.add)
                    else:
                        nc.gpsimd.tensor_copy(u[:], sx[:])
                    mean_t = sm_p.tile([128, 1], f32, tag=f"mean{t}")
                    nc.gpsimd.tensor_tensor(mean_t[:], ag[:, 0:1],
                                            cfrac_t[:], alu.mult)
                    nc.gpsimd.tensor_tensor(u[:], u[:], cinvN_t[:],
                                            alu.mult)
                    nc.gpsimd.tensor_tensor(mean_t[:], mean_t[:], u[:],
                                            alu.add)
                    v = sm_p.tile([128, 1], f32, tag=f"v{t}")
                    if nact > 1:
                        nc.gpsimd.tensor_tensor(v[:], sq[:, 0:1],
                                                sq[:, 1:2], alu.add)
                        for k in range(2, nact):
                            nc.gpsimd.tensor_tensor(v[:], v[:],
                                                    sq[:, k:k + 1], alu.add)
                    else:
                        nc.gpsimd.tensor_copy(v[:], sq[:])
                    s2_t = sm_p.tile([128, 1], f32, tag=f"s2{t}")
                    nc.gpsimd.tensor_tensor(s2_t[:], ag[:, 0:1], ag[:, 0:1],
                                            alu.mult)
                    nc.gpsimd.tensor_tensor(s2_t[:], s2_t[:], ag[:, 1:2],
                                            alu.add)
                    nc.gpsimd.tensor_tensor(s2_t[:], s2_t[:], cfrac_t[:],
                                            alu.mult)
                    nc.gpsimd.tensor_tensor(v[:], v[:], cinvN_t[:],
                                            alu.mult)
                    nc.gpsimd.tensor_tensor(s2_t[:], s2_t[:], v[:],
                                            alu.add)
                    nc.tensor.matmul(ps_st[:, t:t + 1], sel_t[:], mean_t[:],
                                     start=True, stop=True)
                    nc.tensor.matmul(ps_st[:, 4 + t:5 + t], sel_t[:],
                                     s2_t[:], start=True, stop=True)
                st_sb = sm_p.tile([8, 8], f32, tag="st_sb")
                nc.vector.tensor_copy(st_sb[:], ps_st[:])
                mean = sm_p.tile([8, 4], f32, tag="mean")
                nc.vector.tensor_scalar(mean[:], st_sb[:, 0:4],
                                        1.0 / GS, None, op0=alu.mult)
                msq = sm_p.tile([8, 4], f32, tag="msq")
                nc.vector.tensor_scalar(msq[:], st_sb[:, 4:8],
                                        1.0 / GS, None, op0=alu.mult)
                var = sm_p.tile([8, 4], f32, tag="var")
                nc.vector.tensor_tensor(var[:], mean[:], mean[:], alu.mult)
                nc.vector.tensor_tensor(var[:], msq[:], var[:], alu.subtract)
                nc.vector.tensor_scalar(var[:], var[:], EPS, None, op0=alu.add)
                sd = sm_p.tile([8, 4], f32, tag="sd")
                nc.scalar.activation(sd[:], var[:], AF.Sqrt)
                rstd = sm_p.tile([8, 4], f32, tag="rstd")
                nc.vector.reciprocal(rstd[:], sd[:])
                A_t, A16_t, Ai16_t, Bb_t = [], [], [], []
                for t in range(CT):
                    ps_ab = ab_ps.tile([128, 2], f32, tag="ab")
                    nc.tensor.matmul(ps_ab[:, 0:1], selT_t[:],
                                     rstd[:, t:t + 1], start=True, stop=True)
                    nc.tensor.matmul(ps_ab[:, 1:2], selT_t[:],
                                     mean[:, t:t + 1], start=True, stop=True)
                    ab = cpool.tile([128, 2], f32, tag=f"ab{t}")
                    nc.vector.tensor_copy(ab[:], ps_ab[:])
                    At = cpool.tile([128, 1], f32, tag=f"A{t}")
                    nc.vector.tensor_tensor(At[:], ab[:, 0:1], gam_t[t],
                                            alu.mult)
                    At16 = cpool.tile([128, 1], f32, tag=f"A16_{t}")
                    nc.vector.tensor_scalar(At16[:], At[:], WVS, None,
                                            op0=alu.mult)
                    Ai16 = cpool.tile([128, 1], f32, tag=f"Ai16_{t}")
                    nc.vector.tensor_scalar(Ai16[:], At[:], 1.0 / WVS, None,
                                            op0=alu.mult)
                    Bt = cpool.tile([128, 1], f32, tag=f"B{t}")
                    nc.vector.tensor_tensor(Bt[:], ab[:, 1:2], At[:], alu.mult)
                    nc.vector.tensor_tensor(Bt[:], bet_t[t], Bt[:],
                                            alu.subtract)
                    Bb = cpool.tile([128, 1], bf16, tag=f"Bb{t}")
                    nc.vector.tensor_copy(Bb[:], Bt[:])
                    A_t.append(At)
                    A16_t.append(At16)
                    Ai16_t.append(Ai16)
                    Bb_t.append(Bb)

                # fp8 DR weight tiles (prescaled x16; t = 2g + i), on DVE:
                #   wv8 = fp8(A16*wv rows), m08 = fp8(A16*m0 rows)
                # (wp8 is converted later -- it is only needed at the
                # finalize, and converting it here queues DVE work ahead
                # of the q8 writes that gate the first scores)
                wv8_t, m08_t = [], []
                for g in range(NG):
                    w8 = w_p.tile([128, 2, C], f8, tag=f"wv8_{g}")
                    m8 = w_p.tile([128, 2, C], f8, tag=f"m08_{g}")
                    for i in range(2):
                        t = 2 * g + i
                        nc.vector.tensor_scalar(w8[:, i, :], wv_t[t],
                                                A16_t[t][:], None,
                                                op0=alu.mult)
                        nc.vector.tensor_scalar(m8[:, i, :], m0_t[t],
                                                A16_t[t][:], None,
                                                op0=alu.mult)
                    wv8_t.append(w8)
                    m08_t.append(m8)

            # ---- post-stats region: one PSUM layout (3 + 3 + small) ----
            with tc.tile_pool(name="mm_ps", bufs=3, space="PSUM") as mm_ps, \
                 tc.tile_pool(name="att_ps", bufs=3, space="PSUM") as att_ps, \
                 tc.tile_pool(name="sm2_ps", bufs=1, space="PSUM") as smp:

                def vprod(half):
                    j0 = half * JQ
                    v8_t = []
                    for jt in range(JT):
                        ps = mm_ps.tile([128, 512], f32, tag="mm")
                        for g in range(NG):
                            mm8(ps[:],
                                x8_t[g][:, :, j0 + jt * 128:
                                        j0 + (jt + 1) * 128],
                                wv8_t[g][:], g == 0, g == NG - 1)
                        kt2, slot = divmod(jt, 2)
                        if slot == 0:
                            vt = v8_p.tile([128, 2, C], f8, tag="v8")
                            v8_t.append(vt)
                        if jt % 4 < 2:
                            nc.scalar.copy(v8_t[kt2][:, slot, :], ps[:])
                        else:
                            nc.vector.tensor_copy(v8_t[kt2][:, slot, :],
                                                  ps[:])
                    return v8_t

                # half-0 V first: only needs wv8 + x8; warms the PE while
                # the bias/q chain resolves
                v8_half0 = vprod(0)

                # bias terms from RAW weights:
                #   qkb = M0@B + wk^T bq (host const);  Abias = A*qkb
                #   tv  = wv@B  (for the projection-bias fold)
                abias_t, tvb_t = [], []
                for co in range(CT):
                    ps_b = smp.tile([128, 2], f32, tag="bb")
                    for ci in range(CT):
                        mm(ps_b[:, 0:1],
                           m0_t[ci][:, co * 128:(co + 1) * 128], Bb_t[ci][:],
                           ci == 0, ci == CT - 1)
                    for ci in range(CT):
                        mm(ps_b[:, 1:2],
                           wv_t[ci][:, co * 128:(co + 1) * 128], Bb_t[ci][:],
                           ci == 0, ci == CT - 1)
                    ab2 = cpool.tile([128, 1], f32, tag=f"abias{co}")
                    nc.vector.tensor_tensor(ab2[:], ps_b[:, 0:1],
                                            qkbc_t[co], alu.add)
                    nc.vector.tensor_tensor(ab2[:], ab2[:], A_t[co][:],
                                            alu.mult)
                    abias_t.append(ab2)
                    tvb = cpool.tile([128, 1], bf16, tag=f"tvb{co}")
                    nc.vector.tensor_copy(tvb[:], ps_b[:, 1:2])
                    tvb_t.append(tvb)

                # qk projection (fp8 DR) -> fp8 DR tiles q8[g][:, i, :]
                # psum carries 16*q (m08 prescale); scale back with A/16
                q8_t = [q8_p.tile([128, 2, QS], f8, tag=f"q8_{g}",
                                  name=f"q8_{g}") for g in range(NG)]
                for co in range(CT):
                    g, i = divmod(co, 2)
                    for nn in range(QS // 512):
                        ps = mm_ps.tile([128, 512], f32, tag="mm")
                        for gi in range(NG):
                            mm8(ps[:],
                                m08_t[gi][:, :, co * 128:(co + 1) * 128],
                                x8_t[gi][:, :, nn * 512:(nn + 1) * 512],
                                gi == 0, gi == NG - 1)
                        # psum->fp8 writes split DVE/ACT so the serial
                        # drain doesn't gate the first scores
                        if co % 2 == 0:
                            nc.vector.tensor_scalar(
                                q8_t[g][:, i, nn * 512:(nn + 1) * 512],
                                ps[:], Ai16_t[co][:], abias_t[co][:],
                                op0=alu.mult, op1=alu.add)
                        else:
                            nc.scalar.activation(
                                q8_t[g][:, i, nn * 512:(nn + 1) * 512],
                                ps[:], AF.Identity,
                                bias=abias_t[co][:], scale=Ai16_t[co][:])

                # device projection bias bpd = 16*(wp/16)@tv + bp_eff, and
                # xb = x_residual + bpd so the finalize needs one STT
                bpd_t, xb_t = [], []
                for co in range(CT):
                    ps_u = smp.tile([128, 1], f32, tag="u")
                    for ci in range(CT):
                        mm(ps_u[:], wp_t[ci][:, co * 128:(co + 1) * 128],
                           tvb_t[ci][:], ci == 0, ci == CT - 1)
                    bpd = f_p.tile([128, 1], f32, tag=f"bpd{co}")
                    nc.vector.scalar_tensor_tensor(
                        bpd[:], ps_u[:], WVS, bp_t[co],
                        op0=alu.mult, op1=alu.add)
                    bpd_t.append(bpd)
                    xb = f_p.tile([128, QS], bf16, tag=f"xb{co}")
                    nc.vector.tensor_scalar(xb[:], xq_t[co][:],
                                            bpd[:], None, op0=alu.add)
                    xb_t.append(xb)

                # wp8 = fp8(16 * wp.T) [host sent wp.T/16 -> scale 256];
                # emitted after the q-side DVE work (finalize-only use)
                wp8_t = []
                for g in range(NG):
                    p8 = w_p.tile([128, 2, C], f8, tag=f"wp8_{g}")
                    for i in range(2):
                        t = 2 * g + i
                        nc.vector.tensor_scalar(p8[:, i, :], wp_t[t][:],
                                                WVS * WVS, None,
                                                op0=alu.mult)
                    wp8_t.append(p8)

                # ---- attention over key halves (fp8 DoubleRow) ----
                o_p = outer.enter_context(tc.tile_pool(name="outp", bufs=3))
                rb = f_p.tile([128, QS], f32, tag="rb")
                accn8_t = [f_p.tile([128, 2, QS], f8, tag=f"accn8_{g}",
                                    name=f"accn8_{g}") for g in range(NG)]
                for half in range(NHALF):
                    j0 = half * JQ
                    v8_t = v8_half0 if half == 0 else vprod(1)

                    # scores + exp for BOTH chunks first, so the last
                    # chunk's exps (ACT) overlap the first chunk's attnV
                    e8_c = []
                    for ch in range(NCH):
                        i0 = ch * ICH
                        e8_t = []
                        for jt in range(JT):
                            ps = mm_ps.tile([128, ICH], f32, tag="mm")
                            for g in range(NG):
                                mm8(ps[:],
                                    x8_t[g][:, :, j0 + jt * 128:
                                            j0 + (jt + 1) * 128],
                                    q8_t[g][:, :, i0:i0 + ICH],
                                    g == 0, g == NG - 1)
                            kt2, slot = divmod(jt, 2)
                            if slot == 0:
                                et = e8_p.tile([128, 2, ICH], f8, tag="e8")
                                e8_t.append(et)
                            nc.scalar.activation(e8_t[kt2][:, slot, :],
                                                 ps[:], AF.Exp, scale=SCALE,
                                                 bias=esh_t[:])
                        e8_c.append(e8_t)

                    for ch in range(NCH):
                        i0 = ch * ICH
                        e8_t = e8_c[ch]
                        # denominator: all-ones stationary (every output
                        # partition carries the same key-sum; row 0 used)
                        ps_d = att_ps.tile([128, ICH], f32, tag="att")
                        for kt2 in range(KT2):
                            mm8(ps_d[:], ones8_t[:], e8_t[kt2][:],
                                kt2 == 0, kt2 == KT2 - 1)
                        if half == 0:
                            nc.vector.tensor_copy(den_acc[:, i0:i0 + ICH],
                                                  ps_d[0:1, :])
                        else:
                            nc.vector.tensor_tensor(den_acc[:, i0:i0 + ICH],
                                                    den_acc[:, i0:i0 + ICH],
                                                    ps_d[0:1, :], alu.add)
                            nc.vector.reciprocal(recip[:, i0:i0 + ICH],
                                                 den_acc[:, i0:i0 + ICH])
                            # broadcast early: depends only on the denom
                            nc.gpsimd.partition_broadcast(
                                rb[:, i0:i0 + ICH], recip[:, i0:i0 + ICH])
                        for co in range(CT):
                            ps_a = att_ps.tile([128, ICH], f32, tag="att")
                            for kt2 in range(KT2):
                                mm8(ps_a[:],
                                    v8_t[kt2][:, :, co * 128:(co + 1) * 128],
                                    e8_t[kt2][:], kt2 == 0, kt2 == KT2 - 1)
                            sl = slice(i0, i0 + ICH)
                            if half == 0:
                                nc.vector.tensor_copy(
                                    acc_t[co][:, i0:i0 + ICH], ps_a[:])
                            else:
                                nc.vector.tensor_tensor(
                                    acc_t[co][:, i0:i0 + ICH],
                                    acc_t[co][:, i0:i0 + ICH], ps_a[:],
                                    alu.add)
                                # normalize into the fp8 DR tile right
                                # away so the projection is only one
                                # DVE op behind the last attnV chain
                                g, i = divmod(co, 2)
                                nc.vector.tensor_tensor(
                                    accn8_t[g][:, i, sl],
                                    acc_t[co][:, sl], rb[:, sl], alu.mult)
                        if half == 0:
                            continue
                        # finalize this chunk right away (overlaps the
                        # next chunk's attnV): accn8 = fp8(16*attnout),
                        # proj = DR(wp8, accn8) = 256*out,
                        # out = ps/256 + (x + bpd)
                        for co in range(CT):
                            ps = att_ps.tile([128, 512], f32, tag="att")
                            for g in range(NG):
                                mm8(ps[:],
                                    wp8_t[g][:, :, co * 128:(co + 1) * 128],
                                    accn8_t[g][:, :, sl],
                                    g == 0, g == NG - 1)
                            ot = o_p.tile([128, 512], bf16, tag="o")
                            nc.vector.scalar_tensor_tensor(
                                ot[:], ps[:], 1.0 / (WVS * WVS),
                                xb_t[co][:, sl],
                                op0=alu.mult, op1=alu.add)
                            nc.sync.dma_start(
                                out_d[co * 128:(co + 1) * 128, sl], ot[:])

    nc.compile()
    return nc


def kernel(x, gn_gamma, gn_beta, wq, bq, wk, bk, wv, bv, wp, bp):
    import ml_dtypes
    from concourse import bass_utils

    if "nc" not in _CACHE:
        _CACHE["nc"] = _build()
    nc = _CACHE["nc"]

    x = np.asarray(x, np.float32)
    f = np.float32
    bf = ml_dtypes.bfloat16
    f8 = ml_dtypes.float8_e4m3
    wq32 = np.asarray(wq, f)
    wk32 = np.asarray(wk, f)
    m0T = (wq32.T @ wk32).astype(bf)  # (wk^T wq)^T
    qkbc = (wk32.T @ np.asarray(bq, f)).reshape(C, 1)
    wvT = np.asarray(wv, f).T.astype(bf)
    wts2 = np.ascontiguousarray(np.concatenate([m0T, wvT], axis=1))
    wpd = np.ascontiguousarray((np.asarray(wp, f).T / WVS).astype(bf))
    bp_eff = (np.asarray(bp, f)
              + np.asarray(wp, f) @ np.asarray(bv, f)).reshape(C, 1)
    consts4 = np.ascontiguousarray(np.concatenate([
        np.asarray(gn_gamma, f).reshape(C, 1),
        np.asarray(gn_beta, f).reshape(C, 1),
        bp_eff, qkbc], axis=1))
    sel = np.zeros((128, 8), f)
    for p in range(128):
        sel[p, p // GS] = 1.0
    common = {
        "wts2": wts2, "wpd": wpd, "consts4": consts4,
        "sel": sel, "selT": np.ascontiguousarray(sel.T),
        "ones8": np.ones((128, 2, 128), f8),
    }
    in_maps = []
    for core in range(NCORES):
        b, s = divmod(core, 4)
        xb = x[b].reshape(C, N)
        # roll so this core's query block occupies columns 0:QS; key order
        # is permuted identically for all key-side tensors, and softmax
        # sums are order-invariant, so the program is core-independent.
        xperm = np.ascontiguousarray(np.roll(xb, -s * QS, axis=1))
        im = {**common, "xq": xperm[:, 0:QS].astype(bf)}
        for g in range(NG):
            x8g = xperm[g * 256:(g + 1) * 256].reshape(2, 128, N)
            im[f"x8_{g}"] = np.ascontiguousarray(
                x8g.transpose(1, 0, 2)).astype(f8)
        in_maps.append(im)

    res = bass_utils.run_bass_kernel_spmd(nc, in_maps,
                                          core_ids=list(range(NCORES)))
    _CACHE["last_result"] = res

    out = np.empty((B, C, N), np.float32)
    for core in range(NCORES):
        b, s = divmod(core, 4)
        out[b][:, s * QS:(s + 1) * QS] = np.asarray(
            res.results[core]["out"], np.float32)
    return out.reshape(B, C, H, W)
